# revision 1
# baseline (speedup 1.0000x reference)
"""Trainium2 Bass kernel for nn_CompressionAugmentedTrainer.

Strategy (8-core SPMD, channel-sharded):
- Shard C=64 channels across 8 cores (8 ch/core, 512 rows/core); W row-sharded
  to match; partial features all-reduced (tiny [320,512]) before the loss tail.
- The compressed view (keep k<T/2) has an EXACTLY sparse circulant kernel
  (delta/2 + 1/T on odd lags), so its feature is derived post-all-reduce as
  0.5*f0 + a parity-sum correction (computed host-side from x and W column
  parity sums) -- no circulant at all.
- The distorted/combined circulant kernels decay ~1/n, so their circulants
  are banded to the KEPT block-diagonals (final-loss rel err ~6e-5, measured
  host-side against the exact pipeline; gate is 2e-2).
- Everything upstream of the feature all-reduce runs in bf16 (measured loss
  rel err 4.5e-5): halves DMA and SBUF. x/n1/n2 are transposed host-side so
  the device does zero PE transposes.
- noisy / combined use linearity: f(x + s*n) = f(x) + f(s*n); n2's scaled
  noise is added into the combined view tile before its feature matmul.
- Feature matmuls pack 2 view-groups of 64 rows per 128-row matmul:
  (x | s1*n1) and (dist | zt + s2*n2), accumulating over (tb, cl) against
  each streamed W chunk (W read once, in bf16).
"""
import numpy as np

B, C, T, D = 64, 64, 4096, 512
N_CORES = 8
CH = C // N_CORES            # 8 channels per core
R = B * CH                   # 512 rows per core
TBS = T // 128               # 32 t blocks
NOISE_STD = 0.02
TEMP = 0.1
NV = 5                       # views
KEPT = (0, 1, 31)            # banded circulant block-diagonals (circ dist<=1)
NK = len(KEPT)
AR_ROWS = 5 * 64             # x, s1n1, dist, comb, corr partials

_NC_CACHE = {}


def _host_consts(freq_start, time_start):
    k = np.arange(T)
    keep3072 = (k < int(T * 0.75)).astype(np.float64)
    fw = int(0.1 * T)
    fmask = np.where((k >= freq_start) & (k < freq_start + fw), 0.1, 1.0)
    tw = int(0.05 * T)
    tmask = np.where((k >= time_start) & (k < time_start + tw), 0.1, 1.0)
    m1s = (keep3072 + keep3072[(-k) % T]) / 2.0

    cs = [np.real(np.fft.ifft(m)) for m in (fmask, m1s * fmask)]

    # circulant lhsT tiles for kept diagonals: kern[v, di, j, i]
    #   = c_v[(128*d + i - j) % T]
    dd = np.asarray(KEPT)[:, None, None]
    jj = np.arange(128)[None, :, None]
    ii = np.arange(128)[None, None, :]
    idx = (128 * dd + ii - jj) % T
    kern = np.stack([c[idx] for c in cs])          # [2, NK, 128, 128]

    # t-mask per-partition columns for every affected tb
    tb_aff = sorted({t // 128 for t in range(time_start, time_start + tw)})
    tcols = np.stack([tmask[tb * 128:(tb + 1) * 128] for tb in tb_aff],
                     axis=1).astype(np.float32)    # [128, n_aff]

    n = NV * B
    maskmat = (np.eye(n, k=1) + np.eye(n, k=-1)).astype(np.float32)
    cnt = maskmat.sum(1, keepdims=True).astype(np.float32)   # [320, 1]
    return kern, tb_aff, tcols, maskmat, cnt


def _build_nc(tb_aff, n_cores, use_collective):
    import concourse.bacc as bacc
    import concourse.mybir as mybir
    import concourse.tile as tile
    from concourse.masks import make_identity

    DT = mybir.dt.float32
    BF = mybir.dt.bfloat16
    F32R = mybir.dt.float32r
    AF = mybir.ActivationFunctionType
    n_aff = len(tb_aff)

    nc = bacc.Bacc("TRN2", target_bir_lowering=False, debug=False,
                   num_devices=n_cores)

    xt_d = nc.dram_tensor("xts", [T, R], BF, kind="ExternalInput").ap()
    n1_d = nc.dram_tensor("n1ts", [T, R], BF, kind="ExternalInput").ap()
    n2_d = nc.dram_tensor("n2ts", [T, R], BF, kind="ExternalInput").ap()
    w_d = nc.dram_tensor("Ws", [CH * T, D], BF, kind="ExternalInput").ap()
    bias_d = nc.dram_tensor("bias", [1, D], DT, kind="ExternalInput").ap()
    kern_d = nc.dram_tensor("kern", [2 * NK, 128, 128], BF,
                            kind="ExternalInput").ap()
    corr_d = nc.dram_tensor("corr_in", [64, D], DT, kind="ExternalInput").ap()
    s1i_d = nc.dram_tensor("s1_in", [1, R], BF, kind="ExternalInput").ap()
    tm_d = nc.dram_tensor("tmaskc", [128, n_aff], DT, kind="ExternalInput").ap()
    mm_d = nc.dram_tensor("maskmat", [NV * B, NV * B], DT,
                          kind="ExternalInput").ap()
    cnt_d = nc.dram_tensor("cnt", [NV * B, 1], DT, kind="ExternalInput").ap()
    out_d = nc.dram_tensor("out_loss", [1, 1], DT, kind="ExternalOutput").ap()

    s2_d = nc.dram_tensor("s2_bounce", [1, R], BF).ap()
    se_d = nc.dram_tensor("se_bounce", [1, R], DT).ap()
    so_d = nc.dram_tensor("so_bounce", [1, R], DT).ap()
    ar_in = nc.dram_tensor("ar_in", [AR_ROWS, D], DT).ap()
    ar_out = nc.dram_tensor("ar_out", [AR_ROWS, D], DT,
                            addr_space="Shared").ap()

    def mmf(out, lhsT, rhs, start, stop):
        nc.tensor.matmul(out, lhsT.bitcast(F32R), rhs.bitcast(F32R),
                         start=start, stop=stop)

    def mmb(out, lhsT, rhs, start, stop):
        nc.tensor.matmul(out, lhsT, rhs, start=start, stop=stop)

    with tile.TileContext(nc) as tc:
      with tc.tile_pool(name="const", bufs=1) as cp:
        kern_sb = cp.tile([128, 2 * NK * 128], BF, tag="kern")
        nc.sync.dma_start(
            kern_sb[:].rearrange("j (g i) -> j g i", i=128),
            kern_d.rearrange("g j i -> j g i"))
        ident = cp.tile([128, 128], DT, tag="ident")
        make_identity(nc, ident[:])
        ones_raw = cp.tile([128, 1], DT, tag="ones_raw")
        nc.vector.memset(ones_raw[:], 1.0)
        ones = cp.tile([128, 1], DT, tag="ones")
        nc.scalar.copy(ones[:].bitcast(F32R), ones_raw[:])
        onesb = cp.tile([128, 1], BF, tag="onesb")
        nc.vector.memset(onesb[:], 1.0)
        tmc = cp.tile([128, n_aff], DT, tag="tmc")
        nc.sync.dma_start(tmc[:], tm_d)
        xt_sb = cp.tile([128, TBS * R], BF, tag="xt")
        zt_sb = cp.tile([128, TBS * R], BF, tag="zt")
        s1b = cp.tile([128, R], BF, tag="s1b")
        s2b = cp.tile([128, R], BF, tag="s2b")
        corr_sb = cp.tile([64, D], DT, tag="corr")

        def kslice(v, di):
            return kern_sb[:, (v * NK + di) * 128:(v * NK + di + 1) * 128]

        def xslice(tb):
            return xt_sb[:, tb * R:(tb + 1) * R]

        def zslice(tb):
            return zt_sb[:, tb * R:(tb + 1) * R]

        with tc.tile_pool(name="fps", bufs=1, space="PSUM") as fps:
            # ---------- Phase A: load xT; s1/corr come precomputed ----
            for tb in range(TBS):
                nc.sync.dma_start(xslice(tb),
                                  xt_d[tb * 128:(tb + 1) * 128, :])
            nc.gpsimd.dma_start(out=s1b[:], in_=s1i_d.to_broadcast((128, R)))
            nc.sync.dma_start(corr_sb[:], corr_d)

            # ---------- Phase B: combined-view banded circulant + stats ------
            with (
                tc.tile_pool(name="pb_sb", bufs=1) as pb,
                tc.tile_pool(name="pb_ps", bufs=1, space="PSUM") as pbps,
            ):
                zsum_ps = pbps.tile([1, R], DT, tag="zsum")
                zss_ps = pbps.tile([1, R], DT, tag="zss")
                for tb in range(TBS):
                    zp = pbps.tile([128, R], DT, tag="circ", bufs=2)
                    for di, d in enumerate(KEPT):
                        mmb(zp[:], kslice(1, di), xslice((tb - d) % TBS),
                            di == 0, di == NK - 1)
                    if tb in tb_aff:
                        nc.vector.tensor_scalar_mul(
                            zslice(tb), zp[:],
                            tmc[:, tb_aff.index(tb):tb_aff.index(tb) + 1])
                    else:
                        nc.scalar.copy(zslice(tb), zp[:])
                    zsq = pb.tile([128, R], BF, tag="zsq", bufs=2)
                    nc.vector.tensor_mul(zsq[:], zslice(tb), zslice(tb))
                    mmb(zsum_ps[:], onesb[:], zslice(tb), tb == 0, tb == TBS - 1)
                    mmb(zss_ps[:], onesb[:], zsq[:], tb == 0, tb == TBS - 1)
                # s2 = 0.02 * sqrt((ss - sum^2/T)/(T-1)) on [1, R]
                zsum = pb.tile([1, R], DT, tag="zsumsb")
                zss = pb.tile([1, R], DT, tag="zsssb")
                nc.scalar.copy(zsum[:], zsum_ps[:])
                nc.scalar.copy(zss[:], zss_ps[:])
                nc.vector.tensor_mul(zsum[:], zsum[:], zsum[:])
                nc.vector.tensor_scalar_mul(zsum[:], zsum[:], -1.0 / T)
                nc.vector.tensor_add(zsum[:], zsum[:], zss[:])
                nc.scalar.activation(zsum[:], zsum[:], AF.Sqrt,
                                     scale=1.0 / (T - 1))
                nc.scalar.mul(zsum[:], zsum[:], NOISE_STD)
                s2c = pb.tile([1, R], BF, tag="s2c")
                nc.scalar.copy(s2c[:], zsum[:])
                nc.sync.dma_start(s2_d, s2c[:])
                nc.gpsimd.dma_start(out=s2b[:], in_=s2_d.to_broadcast((128, R)))

            # ---------- Phase C: main loop (dist circulant + features) -------
            f1_ps = fps.tile([128, D], DT, tag="f1")   # x | s1*n1
            f2_ps = fps.tile([128, D], DT, tag="f2")   # dist | zt + s2*n2
            with (
                tc.tile_pool(name="pc_sb", bufs=1) as pc,
                tc.tile_pool(name="pc_ps", bufs=1, space="PSUM") as pcps,
            ):
                for tb in range(TBS):
                    p1 = pc.tile([128, 2 * D], BF, tag="p1", bufs=2)
                    p2 = pc.tile([128, 2 * D], BF, tag="p2", bufs=2)
                    nc.scalar.copy(p1[:, 0:D], xslice(tb))
                    nt1 = pc.tile([128, R], BF, tag="nt1", bufs=2)
                    nc.sync.dma_start(nt1[:], n1_d[tb * 128:(tb + 1) * 128, :])
                    nc.vector.tensor_mul(p1[:, D:2 * D], nt1[:], s1b[:])
                    zp = pcps.tile([128, R], DT, tag="circ", bufs=2)
                    for di, d in enumerate(KEPT):
                        mmb(zp[:], kslice(0, di), xslice((tb - d) % TBS),
                            di == 0, di == NK - 1)
                    if tb in tb_aff:
                        nc.vector.tensor_scalar_mul(
                            p2[:, 0:D], zp[:],
                            tmc[:, tb_aff.index(tb):tb_aff.index(tb) + 1])
                    else:
                        nc.scalar.copy(p2[:, 0:D], zp[:])
                    nt2 = pc.tile([128, R], BF, tag="nt2", bufs=2)
                    nc.sync.dma_start(nt2[:], n2_d[tb * 128:(tb + 1) * 128, :])
                    tmpn = pc.tile([128, R], BF, tag="tmpn", bufs=2)
                    nc.vector.tensor_mul(tmpn[:], nt2[:], s2b[:])
                    nc.scalar.copy(p2[:, D:2 * D], zslice(tb))
                    nc.vector.tensor_add(p2[:, D:2 * D], p2[:, D:2 * D],
                                         tmpn[:])
                    for cl in range(CH):
                        wch = pc.tile([128, D], BF, tag="w", bufs=4)
                        nc.sync.dma_start(
                            wch[:],
                            w_d[cl * T + tb * 128:cl * T + (tb + 1) * 128, :])
                        st = tb == 0 and cl == 0
                        sp = tb == TBS - 1 and cl == CH - 1
                        for pt, fp in ((p1, f1_ps), (p2, f2_ps)):
                            lhs = pt[:].rearrange("p (v b c) -> p v b c",
                                                  v=2, c=CH)[:, :, :, cl]
                            mmb(fp[:], lhs, wch[:], st, sp)

            # ---------- Phase D: all-reduce partial features ----------
            with tc.tile_pool(name="pd_sb", bufs=1) as pd:
                fsb = [pd.tile([128, D], DT, tag=f"fsb{i}", name=f"fsb{i}")
                       for i in range(2)]
                nc.scalar.copy(fsb[0][:], f1_ps[:])
                nc.scalar.copy(fsb[1][:], f2_ps[:])
                # ar rows: 0:64 x, 64:128 s1n1, 128:192 dist, 192:256 comb,
                #          256:320 corr
                nc.gpsimd.dma_start(ar_in[0:128], fsb[0][:])
                nc.gpsimd.dma_start(ar_in[128:256], fsb[1][:])
                nc.gpsimd.dma_start(ar_in[256:320], corr_sb[:])
                if use_collective:
                    nc.gpsimd.collective_compute(
                        "AllReduce", mybir.AluOpType.add,
                        replica_groups=[list(range(n_cores))],
                        ins=[ar_in], outs=[ar_out])
                else:
                    nc.gpsimd.dma_start(ar_out, ar_in)

        # ---------- Phase E: loss tail (identical on every core) ----------
        with (
            tc.tile_pool(name="pe_sb", bufs=1) as pe,
            tc.tile_pool(name="pe_ps", bufs=1, space="PSUM") as peps,
        ):
            bb = pe.tile([128, D], DT, tag="bb")
            nc.gpsimd.dma_start(out=bb[:], in_=bias_d.to_broadcast((128, D)))
            ag = [pe.tile([64, D], DT, tag=f"ag{v}", name=f"ag{v}")
                  for v in range(5)]
            for v in range(5):
                nc.sync.dma_start(ag[v][:], ar_out[v * 64:(v + 1) * 64])
            # features (dict order): x, compressed, distorted, noisy, combined
            fv = [pe.tile([64, D], DT, tag=f"fv{v}", name=f"fv{v}")
                  for v in range(NV)]
            nc.vector.tensor_add(fv[0][:], ag[0][:], bb[0:64, :])      # x
            nc.vector.tensor_scalar_mul(fv[1][:], ag[0][:], 0.5)       # comp
            nc.vector.tensor_add(fv[1][:], fv[1][:], ag[4][:])
            nc.vector.tensor_add(fv[1][:], fv[1][:], bb[0:64, :])
            nc.vector.tensor_add(fv[2][:], ag[2][:], bb[0:64, :])      # dist
            nc.vector.tensor_add(fv[3][:], ag[0][:], ag[1][:])         # noisy
            nc.vector.tensor_add(fv[3][:], fv[3][:], bb[0:64, :])
            nc.vector.tensor_add(fv[4][:], ag[3][:], bb[0:64, :])      # comb

            # consistency: sum over v of ||f0 - fv||^2
            cacc = pe.tile([64, 4], DT, tag="cacc")
            for v in range(1, NV):
                dd = pe.tile([64, D], DT, tag="dd", bufs=2)
                nc.vector.tensor_sub(dd[:], fv[v][:], fv[0][:])
                dsq = pe.tile([64, D], DT, tag="dsq", bufs=2)
                nc.scalar.activation(dsq[:], dd[:], AF.Square,
                                     accum_out=cacc[:, v - 1:v])
            cps = peps.tile([1, 4], DT, tag="smallps")
            nc.tensor.matmul(cps[:], ones[0:64, :], cacc[:],
                             start=True, stop=True)
            csb = pe.tile([1, 4], DT, tag="csb")
            nc.scalar.copy(csb[:], cps[:])
            cons = pe.tile([1, 1], DT, tag="cons")
            nc.vector.tensor_reduce(cons[:], csb[:], mybir.AxisListType.X,
                                    mybir.AluOpType.add)

            # normalize rows
            for v in range(NV):
                nrm = pe.tile([64, 1], DT, tag="nrm", bufs=2)
                scr = pe.tile([64, D], DT, tag="scr", bufs=2)
                nc.scalar.activation(scr[:], fv[v][:], AF.Square,
                                     accum_out=nrm[:])
                nc.scalar.sqrt(nrm[:], nrm[:])
                rnr = pe.tile([64, 1], DT, tag="rnr", bufs=2)
                nc.vector.reciprocal(rnr[:], nrm[:])
                nc.vector.tensor_scalar_mul(fv[v][:], fv[v][:], rnr[:])

            # fnT [d-part, 320]
            fnT = [pe.tile([128, NV * B], DT, tag=f"fnT{dc}", name=f"fnT{dc}")
                   for dc in range(4)]
            for v in range(NV):
                for dc in range(4):
                    tp = peps.tile([128, 64], DT, tag="ttr", bufs=2)
                    nc.tensor.transpose(
                        tp[:], fv[v][:, dc * 128:(dc + 1) * 128],
                        ident[0:64, 0:64])
                    nc.scalar.copy(fnT[dc][:, v * 64:(v + 1) * 64], tp[:])

            # sim rows, logsumexp, masked sums
            mrow = [0, 128, 256]
            mlen = [128, 128, 64]
            parts = []
            for rk in range(3):
                n_r = mlen[rk]
                sps = peps.tile([n_r, NV * B], DT, tag="sps", bufs=2)
                for dc in range(4):
                    lhs = fnT[dc][:, mrow[rk]:mrow[rk] + n_r]
                    nc.tensor.matmul(sps[:], lhs, fnT[dc][:],
                                     start=dc == 0, stop=dc == 3)
                sim = pe.tile([n_r, NV * B], DT, tag=f"sim{rk}")
                nc.scalar.copy(sim[:], sps[:])
                mx = pe.tile([n_r, 1], DT, tag="mx", bufs=2)
                nc.vector.tensor_reduce(mx[:], sim[:], mybir.AxisListType.X,
                                        mybir.AluOpType.max)
                nm10 = pe.tile([n_r, 1], DT, tag="nm10", bufs=2)
                nc.vector.tensor_scalar_mul(nm10[:], mx[:], -10.0)
                esc = pe.tile([n_r, NV * B], DT, tag="esc", bufs=2)
                sume = pe.tile([n_r, 1], DT, tag="sume", bufs=2)
                nc.scalar.activation(esc[:], sim[:], AF.Exp,
                                     bias=nm10[:], scale=10.0,
                                     accum_out=sume[:])
                lse = pe.tile([n_r, 1], DT, tag="lse", bufs=2)
                nc.scalar.activation(lse[:], sume[:], AF.Ln)
                m10 = pe.tile([n_r, 1], DT, tag="m10", bufs=2)
                nc.vector.tensor_scalar_mul(m10[:], mx[:], 10.0)
                nc.vector.tensor_add(lse[:], lse[:], m10[:])
                # masked raw sum
                mmt = pe.tile([n_r, NV * B], DT, tag="mmt", bufs=2)
                nc.sync.dma_start(mmt[:], mm_d[mrow[rk]:mrow[rk] + n_r, :])
                nc.vector.tensor_mul(mmt[:], mmt[:], sim[:])
                mr = pe.tile([n_r, 1], DT, tag="mr", bufs=2)
                nc.vector.tensor_reduce(mr[:], mmt[:], mybir.AxisListType.X,
                                        mybir.AluOpType.add)
                nc.vector.tensor_scalar_mul(mr[:], mr[:], 10.0)
                cntt = pe.tile([n_r, 1], DT, tag="cntt", bufs=2)
                nc.sync.dma_start(cntt[:], cnt_d[mrow[rk]:mrow[rk] + n_r, :])
                nc.vector.tensor_mul(cntt[:], cntt[:], lse[:])
                nc.vector.tensor_sub(mr[:], mr[:], cntt[:])
                parts.append(mr)
            stk = pe.tile([128, 3], DT, tag="stk")
            nc.vector.memset(stk[:], 0.0)
            nc.scalar.copy(stk[:, 0:1], parts[0][:])
            nc.scalar.copy(stk[:, 1:2], parts[1][:])
            nc.scalar.copy(stk[0:64, 2:3], parts[2][:])
            mps = peps.tile([1, 3], DT, tag="smallps")
            nc.tensor.matmul(mps[:], ones[:], stk[:], start=True, stop=True)
            msb = pe.tile([1, 3], DT, tag="msb")
            nc.scalar.copy(msb[:], mps[:])
            msum = pe.tile([1, 1], DT, tag="msum")
            nc.vector.tensor_reduce(msum[:], msb[:], mybir.AxisListType.X,
                                    mybir.AluOpType.add)

            # total = cons/(4*B*D) - 0.5 * msum / (2*NV*B - 2)
            nc.scalar.mul(cons[:], cons[:], 1.0 / (4 * B * D))
            nc.scalar.mul(msum[:], msum[:], -0.5 / float(2 * NV * B - 2))
            tot = pe.tile([1, 1], DT, tag="tot")
            nc.vector.tensor_add(tot[:], cons[:], msum[:])
            nc.sync.dma_start(out_d, tot[:])

    nc.compile()
    return nc


def _get_nc(tb_aff, n_cores, use_collective):
    key = (tuple(tb_aff), n_cores, use_collective)
    if key not in _NC_CACHE:
        _NC_CACHE[key] = _build_nc(list(tb_aff), n_cores, use_collective)
    return _NC_CACHE[key]


def make_in_maps(x, W, b, noise1, noise2, freq_start, time_start):
    import ml_dtypes
    BF16 = ml_dtypes.bfloat16
    kern, tb_aff, tcols, maskmat, cnt = _host_consts(
        int(freq_start), int(time_start))
    x = np.asarray(x, dtype=np.float32)
    W = np.asarray(W, dtype=np.float32)
    b = np.asarray(b, dtype=np.float32)
    xbf = x.astype(BF16)
    n1bf = np.asarray(noise1, dtype=np.float32).astype(BF16)
    n2bf = np.asarray(noise2, dtype=np.float32).astype(BF16)
    Wr = W.reshape(C, T, D)
    Wbf = W.astype(BF16).reshape(C, T, D)
    # parity column sums of W (fp32), scaled by 1/T
    WeT = Wr[:, 0::2, :].sum(axis=1) * (1.0 / T)   # [C, D], pairs with s_odd
    WoT = Wr[:, 1::2, :].sum(axis=1) * (1.0 / T)   # pairs with s_even
    kern_bf = kern.reshape(2 * NK, 128, 128).astype(BF16)
    in_maps = []
    for core in range(N_CORES):
        cs = core * CH
        xr = x[:, cs:cs + CH, :].reshape(R, T)
        s1v = (NOISE_STD * xr.std(-1, ddof=1)).astype(BF16).reshape(1, R)
        s_e = xr[:, 0::2].sum(-1).reshape(B, CH)
        s_o = xr[:, 1::2].sum(-1).reshape(B, CH)
        corr = (s_o @ WeT[cs:cs + CH] + s_e @ WoT[cs:cs + CH]).astype(
            np.float32)
        in_maps.append({
            "xts": np.ascontiguousarray(
                xbf[:, cs:cs + CH, :].reshape(R, T).T),
            "n1ts": np.ascontiguousarray(
                n1bf[:, cs:cs + CH, :].reshape(R, T).T),
            "n2ts": np.ascontiguousarray(
                n2bf[:, cs:cs + CH, :].reshape(R, T).T),
            "Ws": np.ascontiguousarray(
                Wbf[cs:cs + CH].reshape(CH * T, D)),
            "bias": b.reshape(1, D),
            "kern": kern_bf,
            "corr_in": corr,
            "s1_in": s1v,
            "tmaskc": tcols,
            "maskmat": maskmat,
            "cnt": cnt,
        })
    return in_maps, tb_aff


def kernel(x, W, b, noise1, noise2, freq_start, time_start):
    from concourse.bass_utils import run_bass_kernel_spmd
    in_maps, tb_aff = make_in_maps(x, W, b, noise1, noise2,
                                   freq_start, time_start)
    nc = _get_nc(tb_aff, N_CORES, True)
    res = run_bass_kernel_spmd(nc, in_maps, core_ids=list(range(N_CORES)))
    return np.float32(res.results[0]["out_loss"].reshape(())[()])



# revision 2
# speedup vs baseline: 1.8431x; 1.8431x over previous
"""Trainium2 Bass kernel for nn_CompressionAugmentedTrainer.

Strategy (8-core SPMD, channel-sharded):
- Shard C=64 channels across 8 cores (8 ch/core, 512 rows/core); W row-sharded
  to match; partial features all-reduced (tiny [256,512]) before the loss tail.
- The compressed view (keep k<T/2) has an EXACTLY sparse circulant kernel
  (delta/2 + 1/T on odd lags), so its feature is derived post-all-reduce as
  0.5*f0 + a parity-sum correction (host-side from x and W column parity
  sums, summed across cores) -- no circulant at all.
- The distorted/combined circulant kernels decay ~1/n; banded to the single
  block-diagonal (128-wide) of the circulant (final-loss rel err 8.3e-5
  measured host-side against the exact pipeline; gate is 2e-2).
- s2 (noise scale of the combined view) is computed host-side via FFT, so
  the kernel is ONE fused loop over t-blocks: banded circulants for
  dist/comb + packed feature matmuls, no separate stats pass.
- Everything upstream of the feature all-reduce runs in bf16: halves DMA
  and SBUF. x/n1 are packed host-side into one [T, 2R] tensor so each
  t-block needs only 3 DMAs (x|n1, n2, W); W is pre-transposed host-side to
  [T, CH*D] so one contiguous 1MB DMA per t-block feeds all 8 channels.
- noisy / combined use linearity: f(x + s*n) = f(x) + f(s*n); feature
  matmuls pack 2 view-groups of 64 rows per 128-row matmul:
  (x | s1*n1) and (dist | zt + s2*n2), accumulating over (tb, cl) against
  each streamed W chunk (W read once, in bf16).
"""
import numpy as np

B, C, T, D = 64, 64, 4096, 512
N_CORES = 8
CH = C // N_CORES            # 8 channels per core
R = B * CH                   # 512 rows per core
TBS = T // 128               # 32 t blocks
NOISE_STD = 0.02
TEMP = 0.1
NV = 5                       # views
AR_ROWS = 4 * 64             # x, s1n1, dist, comb partials

_NC_CACHE = {}


def _host_consts(freq_start, time_start):
    k = np.arange(T)
    keep3072 = (k < int(T * 0.75)).astype(np.float64)
    fw = int(0.1 * T)
    fmask = np.where((k >= freq_start) & (k < freq_start + fw), 0.1, 1.0)
    tw = int(0.05 * T)
    tmask = np.where((k >= time_start) & (k < time_start + tw), 0.1, 1.0)
    m1s = (keep3072 + keep3072[(-k) % T]) / 2.0

    cs = [np.real(np.fft.ifft(m)) for m in (fmask, m1s * fmask)]

    # single block-diagonal circulant lhsT tiles: kern[v, j, i] = c_v[(i-j)%T]
    jj = np.arange(128)[:, None]
    ii = np.arange(128)[None, :]
    idx = (ii - jj) % T
    kern = np.stack([c[idx] for c in cs])          # [2, 128, 128]

    # t-mask per-partition columns for every affected tb
    tb_aff = sorted({t // 128 for t in range(time_start, time_start + tw)})
    tcols = np.stack([tmask[tb * 128:(tb + 1) * 128] for tb in tb_aff],
                     axis=1).astype(np.float32)    # [128, n_aff]

    n = NV * B
    maskmat = (np.eye(n, k=1) + np.eye(n, k=-1)).astype(np.float32)
    cnt = maskmat.sum(1, keepdims=True).astype(np.float32)   # [320, 1]

    # symmetrized spectral mask for host-side s2 (rfft half-spectrum)
    fmask_s = (fmask + fmask[(-k) % T]) / 2.0
    msym_half = (m1s * fmask_s)[:T // 2 + 1]
    return kern, tb_aff, tcols, maskmat, cnt, tmask, msym_half


def _build_nc(tb_aff, n_cores, use_collective):
    import concourse.bacc as bacc
    import concourse.mybir as mybir
    import concourse.tile as tile
    from concourse.masks import make_identity

    DT = mybir.dt.float32
    BF = mybir.dt.bfloat16
    F32R = mybir.dt.float32r
    AF = mybir.ActivationFunctionType
    n_aff = len(tb_aff)

    nc = bacc.Bacc("TRN2", target_bir_lowering=False, debug=False,
                   num_devices=n_cores)

    xn1_d = nc.dram_tensor("xn1s", [T, 2 * R], BF, kind="ExternalInput").ap()
    n2_d = nc.dram_tensor("n2ts", [T, R], BF, kind="ExternalInput").ap()
    w_d = nc.dram_tensor("Ws", [T, CH * D], BF, kind="ExternalInput").ap()
    bias_d = nc.dram_tensor("bias", [1, D], DT, kind="ExternalInput").ap()
    kern_d = nc.dram_tensor("kern", [2, 128, 128], BF,
                            kind="ExternalInput").ap()
    corr_d = nc.dram_tensor("corr_in", [64, D], DT, kind="ExternalInput").ap()
    s1i_d = nc.dram_tensor("s1_in", [1, R], BF, kind="ExternalInput").ap()
    s2i_d = nc.dram_tensor("s2_in", [1, R], BF, kind="ExternalInput").ap()
    tm_d = nc.dram_tensor("tmaskc", [128, n_aff], DT, kind="ExternalInput").ap()
    mm_d = nc.dram_tensor("maskmat", [NV * B, NV * B], DT,
                          kind="ExternalInput").ap()
    cnt_d = nc.dram_tensor("cnt", [NV * B, 1], DT, kind="ExternalInput").ap()
    out_d = nc.dram_tensor("out_loss", [1, 1], DT, kind="ExternalOutput").ap()

    ar_in = nc.dram_tensor("ar_in", [AR_ROWS, D], DT).ap()
    ar_out = nc.dram_tensor("ar_out", [AR_ROWS, D], DT,
                            addr_space="Shared").ap()

    def mmb(out, lhsT, rhs, start, stop):
        nc.tensor.matmul(out, lhsT, rhs, start=start, stop=stop)

    with tile.TileContext(nc) as tc:
      with tc.tile_pool(name="const", bufs=1) as cp:
        kern_sb = cp.tile([128, 2 * 128], BF, tag="kern")
        nc.sync.dma_start(
            kern_sb[:].rearrange("j (g i) -> j g i", i=128),
            kern_d.rearrange("g j i -> j g i"))
        ident = cp.tile([128, 128], DT, tag="ident")
        make_identity(nc, ident[:])
        ones_raw = cp.tile([128, 1], DT, tag="ones_raw")
        nc.vector.memset(ones_raw[:], 1.0)
        ones = cp.tile([128, 1], DT, tag="ones")
        nc.scalar.copy(ones[:].bitcast(F32R), ones_raw[:])
        tmc = cp.tile([128, n_aff], DT, tag="tmc")
        nc.sync.dma_start(tmc[:], tm_d)
        s1b = cp.tile([128, R], BF, tag="s1b")
        s2b = cp.tile([128, R], BF, tag="s2b")
        corr_sb = cp.tile([64, D], DT, tag="corr")

        def kslice(v):
            return kern_sb[:, v * 128:(v + 1) * 128]

        with tc.tile_pool(name="fps", bufs=1, space="PSUM") as fps:
            nc.gpsimd.dma_start(out=s1b[:], in_=s1i_d.to_broadcast((128, R)))
            nc.gpsimd.dma_start(out=s2b[:], in_=s2i_d.to_broadcast((128, R)))
            nc.sync.dma_start(corr_sb[:], corr_d)

            # ---------- fused main loop: circulants + features ----------
            f1_ps = fps.tile([128, D], DT, tag="f1")   # x | s1*n1
            f2_ps = fps.tile([128, D], DT, tag="f2")   # dist | zt + s2*n2
            with (
                tc.tile_pool(name="pc_sb", bufs=1) as pc,
                tc.tile_pool(name="pc_ps", bufs=1, space="PSUM") as pcps,
            ):
                for tb in range(TBS):
                    px = pc.tile([128, 2 * R], BF, tag="px", bufs=3)
                    nc.sync.dma_start(px[:],
                                      xn1_d[tb * 128:(tb + 1) * 128, :])
                    nc.vector.tensor_mul(px[:, R:2 * R], px[:, R:2 * R],
                                         s1b[:])
                    xsl = px[:, 0:R]
                    # combined-view banded circulant
                    zc = pcps.tile([128, R], DT, tag="zc", bufs=2)
                    mmb(zc[:], kslice(1), xsl, True, True)
                    ztv = pc.tile([128, R], BF, tag="ztv", bufs=2)
                    if tb in tb_aff:
                        a = tb_aff.index(tb)
                        nc.vector.tensor_scalar_mul(ztv[:], zc[:],
                                                    tmc[:, a:a + 1])
                    else:
                        nc.scalar.copy(ztv[:], zc[:])
                    pz = pc.tile([128, 2 * R], BF, tag="pz", bufs=3)
                    nc.sync.dma_start(pz[:, R:2 * R],
                                      n2_d[tb * 128:(tb + 1) * 128, :])
                    nc.vector.tensor_mul(pz[:, R:2 * R], pz[:, R:2 * R],
                                         s2b[:])
                    nc.vector.tensor_add(pz[:, R:2 * R], pz[:, R:2 * R],
                                         ztv[:])
                    # distorted-view banded circulant
                    zd = pcps.tile([128, R], DT, tag="zd", bufs=2)
                    mmb(zd[:], kslice(0), xsl, True, True)
                    if tb in tb_aff:
                        a = tb_aff.index(tb)
                        nc.vector.tensor_scalar_mul(pz[:, 0:R], zd[:],
                                                    tmc[:, a:a + 1])
                    else:
                        nc.scalar.copy(pz[:, 0:R], zd[:])
                    # feature matmuls against streamed W chunk
                    wb = pc.tile([128, CH * D], BF, tag="w", bufs=2)
                    nc.sync.dma_start(wb[:],
                                      w_d[tb * 128:(tb + 1) * 128, :])
                    for cl in range(CH):
                        st = tb == 0 and cl == 0
                        sp = tb == TBS - 1 and cl == CH - 1
                        wsl = wb[:, cl * D:(cl + 1) * D]
                        for pt, fp in ((px, f1_ps), (pz, f2_ps)):
                            lhs = pt[:].rearrange("p (v b c) -> p v b c",
                                                  v=2, c=CH)[:, :, :, cl]
                            mmb(fp[:], lhs, wsl, st, sp)

            # ---------- all-reduce partial features ----------
            with tc.tile_pool(name="pd_sb", bufs=1) as pd:
                fsb = [pd.tile([128, D], DT, tag=f"fsb{i}", name=f"fsb{i}")
                       for i in range(2)]
                nc.scalar.copy(fsb[0][:], f1_ps[:])
                nc.scalar.copy(fsb[1][:], f2_ps[:])
                # ar rows: 0:64 x, 64:128 s1n1, 128:192 dist, 192:256 comb
                nc.gpsimd.dma_start(ar_in[0:128], fsb[0][:])
                nc.gpsimd.dma_start(ar_in[128:256], fsb[1][:])
                if use_collective:
                    nc.gpsimd.collective_compute(
                        "AllReduce", mybir.AluOpType.add,
                        replica_groups=[list(range(n_cores))],
                        ins=[ar_in], outs=[ar_out])
                else:
                    nc.gpsimd.dma_start(ar_out, ar_in)

        # ---------- loss tail (identical on every core) ----------
        with (
            tc.tile_pool(name="pe_sb", bufs=1) as pe,
            tc.tile_pool(name="pe_ps", bufs=1, space="PSUM") as peps,
        ):
            bb = pe.tile([128, D], DT, tag="bb")
            nc.gpsimd.dma_start(out=bb[:], in_=bias_d.to_broadcast((128, D)))
            ag = [pe.tile([64, D], DT, tag=f"ag{v}", name=f"ag{v}")
                  for v in range(4)]
            for v in range(4):
                nc.sync.dma_start(ag[v][:], ar_out[v * 64:(v + 1) * 64])
            # features (dict order): x, compressed, distorted, noisy, combined
            fv = [pe.tile([64, D], DT, tag=f"fv{v}", name=f"fv{v}")
                  for v in range(NV)]
            nc.vector.tensor_add(fv[0][:], ag[0][:], bb[0:64, :])      # x
            nc.vector.tensor_scalar_mul(fv[1][:], ag[0][:], 0.5)       # comp
            nc.vector.tensor_add(fv[1][:], fv[1][:], corr_sb[:])
            nc.vector.tensor_add(fv[1][:], fv[1][:], bb[0:64, :])
            nc.vector.tensor_add(fv[2][:], ag[2][:], bb[0:64, :])      # dist
            nc.vector.tensor_add(fv[3][:], ag[0][:], ag[1][:])         # noisy
            nc.vector.tensor_add(fv[3][:], fv[3][:], bb[0:64, :])
            nc.vector.tensor_add(fv[4][:], ag[3][:], bb[0:64, :])      # comb

            # consistency: sum over v of ||f0 - fv||^2
            cacc = pe.tile([64, 4], DT, tag="cacc")
            for v in range(1, NV):
                dd = pe.tile([64, D], DT, tag="dd", bufs=2)
                nc.vector.tensor_sub(dd[:], fv[v][:], fv[0][:])
                dsq = pe.tile([64, D], DT, tag="dsq", bufs=2)
                nc.scalar.activation(dsq[:], dd[:], AF.Square,
                                     accum_out=cacc[:, v - 1:v])
            cps = peps.tile([1, 4], DT, tag="smallps")
            nc.tensor.matmul(cps[:], ones[0:64, :], cacc[:],
                             start=True, stop=True)
            csb = pe.tile([1, 4], DT, tag="csb")
            nc.scalar.copy(csb[:], cps[:])
            cons = pe.tile([1, 1], DT, tag="cons")
            nc.vector.tensor_reduce(cons[:], csb[:], mybir.AxisListType.X,
                                    mybir.AluOpType.add)

            # normalize rows
            for v in range(NV):
                nrm = pe.tile([64, 1], DT, tag="nrm", bufs=2)
                scr = pe.tile([64, D], DT, tag="scr", bufs=2)
                nc.scalar.activation(scr[:], fv[v][:], AF.Square,
                                     accum_out=nrm[:])
                nc.scalar.sqrt(nrm[:], nrm[:])
                rnr = pe.tile([64, 1], DT, tag="rnr", bufs=2)
                nc.vector.reciprocal(rnr[:], nrm[:])
                nc.vector.tensor_scalar_mul(fv[v][:], fv[v][:], rnr[:])

            # fnT [d-part, 320]
            fnT = [pe.tile([128, NV * B], DT, tag=f"fnT{dc}", name=f"fnT{dc}")
                   for dc in range(4)]
            for v in range(NV):
                for dc in range(4):
                    tp = peps.tile([128, 64], DT, tag="ttr", bufs=2)
                    nc.tensor.transpose(
                        tp[:], fv[v][:, dc * 128:(dc + 1) * 128],
                        ident[0:64, 0:64])
                    nc.scalar.copy(fnT[dc][:, v * 64:(v + 1) * 64], tp[:])

            # sim rows, logsumexp (no max shift: |sim/T|<=10), masked sums
            mrow = [0, 128, 256]
            mlen = [128, 128, 64]
            parts = []
            for rk in range(3):
                n_r = mlen[rk]
                sps = peps.tile([n_r, NV * B], DT, tag="sps", bufs=2)
                for dc in range(4):
                    lhs = fnT[dc][:, mrow[rk]:mrow[rk] + n_r]
                    nc.tensor.matmul(sps[:], lhs, fnT[dc][:],
                                     start=dc == 0, stop=dc == 3)
                sim = pe.tile([n_r, NV * B], DT, tag=f"sim{rk}")
                nc.scalar.copy(sim[:], sps[:])
                esc = pe.tile([n_r, NV * B], DT, tag="esc", bufs=2)
                sume = pe.tile([n_r, 1], DT, tag="sume", bufs=2)
                nc.scalar.activation(esc[:], sim[:], AF.Exp,
                                     scale=10.0, accum_out=sume[:])
                lse = pe.tile([n_r, 1], DT, tag="lse", bufs=2)
                nc.scalar.activation(lse[:], sume[:], AF.Ln)
                # masked raw sum
                mmt = pe.tile([n_r, NV * B], DT, tag="mmt", bufs=2)
                nc.sync.dma_start(mmt[:], mm_d[mrow[rk]:mrow[rk] + n_r, :])
                nc.vector.tensor_mul(mmt[:], mmt[:], sim[:])
                mr = pe.tile([n_r, 1], DT, tag="mr", bufs=2)
                nc.vector.tensor_reduce(mr[:], mmt[:], mybir.AxisListType.X,
                                        mybir.AluOpType.add)
                nc.vector.tensor_scalar_mul(mr[:], mr[:], 10.0)
                cntt = pe.tile([n_r, 1], DT, tag="cntt", bufs=2)
                nc.sync.dma_start(cntt[:], cnt_d[mrow[rk]:mrow[rk] + n_r, :])
                nc.vector.tensor_mul(cntt[:], cntt[:], lse[:])
                nc.vector.tensor_sub(mr[:], mr[:], cntt[:])
                parts.append(mr)
            stk = pe.tile([128, 3], DT, tag="stk")
            nc.vector.memset(stk[:], 0.0)
            nc.scalar.copy(stk[:, 0:1], parts[0][:])
            nc.scalar.copy(stk[:, 1:2], parts[1][:])
            nc.scalar.copy(stk[0:64, 2:3], parts[2][:])
            mps = peps.tile([1, 3], DT, tag="smallps")
            nc.tensor.matmul(mps[:], ones[:], stk[:], start=True, stop=True)
            msb = pe.tile([1, 3], DT, tag="msb")
            nc.scalar.copy(msb[:], mps[:])
            msum = pe.tile([1, 1], DT, tag="msum")
            nc.vector.tensor_reduce(msum[:], msb[:], mybir.AxisListType.X,
                                    mybir.AluOpType.add)

            # total = cons/(4*B*D) - 0.5 * msum / (2*NV*B - 2)
            nc.scalar.mul(cons[:], cons[:], 1.0 / (4 * B * D))
            nc.scalar.mul(msum[:], msum[:], -0.5 / float(2 * NV * B - 2))
            tot = pe.tile([1, 1], DT, tag="tot")
            nc.vector.tensor_add(tot[:], cons[:], msum[:])
            nc.sync.dma_start(out_d, tot[:])

    nc.compile()
    return nc


def _get_nc(tb_aff, n_cores, use_collective):
    key = (tuple(tb_aff), n_cores, use_collective)
    if key not in _NC_CACHE:
        _NC_CACHE[key] = _build_nc(list(tb_aff), n_cores, use_collective)
    return _NC_CACHE[key]


def make_in_maps(x, W, b, noise1, noise2, freq_start, time_start):
    import ml_dtypes
    BF16 = ml_dtypes.bfloat16
    kern, tb_aff, tcols, maskmat, cnt, tmask, msym_half = _host_consts(
        int(freq_start), int(time_start))
    x = np.asarray(x, dtype=np.float32)
    W = np.asarray(W, dtype=np.float32)
    b = np.asarray(b, dtype=np.float32)
    xbf = x.astype(BF16)
    n1bf = np.asarray(noise1, dtype=np.float32).astype(BF16)
    n2bf = np.asarray(noise2, dtype=np.float32).astype(BF16)

    # host-side noise scales: s1 from x, s2 from the exact combined view
    z = np.fft.irfft(np.fft.rfft(x, axis=-1) * msym_half, axis=-1)
    zt = z * tmask
    s1_all = (NOISE_STD * x.std(-1, ddof=1)).astype(BF16)       # [B, C]
    s2_all = (NOISE_STD * zt.std(-1, ddof=1)).astype(BF16)      # [B, C]

    # exact compressed-view correction, summed over all channels (fp32)
    Wr = W.reshape(C, T, D)
    WeT = Wr[:, 0::2, :].sum(axis=1) * (1.0 / T)   # [C, D]
    WoT = Wr[:, 1::2, :].sum(axis=1) * (1.0 / T)
    s_e = x[:, :, 0::2].sum(-1)                    # [B, C]
    s_o = x[:, :, 1::2].sum(-1)
    corr_tot = (s_o @ WeT + s_e @ WoT).astype(np.float32)       # [B, D]

    Wbf = W.astype(BF16).reshape(C, T, D)
    kern_bf = kern.astype(BF16)
    in_maps = []
    for core in range(N_CORES):
        cs = core * CH
        xts = xbf[:, cs:cs + CH, :].reshape(R, T).T
        n1ts = n1bf[:, cs:cs + CH, :].reshape(R, T).T
        in_maps.append({
            "xn1s": np.ascontiguousarray(
                np.concatenate([xts, n1ts], axis=1)),
            "n2ts": np.ascontiguousarray(
                n2bf[:, cs:cs + CH, :].reshape(R, T).T),
            "Ws": np.ascontiguousarray(
                Wbf[cs:cs + CH].transpose(1, 0, 2).reshape(T, CH * D)),
            "bias": b.reshape(1, D),
            "kern": kern_bf,
            "corr_in": corr_tot,
            "s1_in": s1_all[:, cs:cs + CH].reshape(1, R),
            "s2_in": s2_all[:, cs:cs + CH].reshape(1, R),
            "tmaskc": tcols,
            "maskmat": maskmat,
            "cnt": cnt,
        })
    return in_maps, tb_aff


def kernel(x, W, b, noise1, noise2, freq_start, time_start):
    from concourse.bass_utils import run_bass_kernel_spmd
    in_maps, tb_aff = make_in_maps(x, W, b, noise1, noise2,
                                   freq_start, time_start)
    nc = _get_nc(tb_aff, N_CORES, True)
    res = run_bass_kernel_spmd(nc, in_maps, core_ids=list(range(N_CORES)))
    return np.float32(res.results[0]["out_loss"].reshape(())[()])


# revision 6
# speedup vs baseline: 2.7850x; 1.5110x over previous
"""Trainium2 Bass kernel for nn_CompressionAugmentedTrainer.

Strategy (SPMD, channel-sharded across N_CORES cores):
- Shard C=64 channels across cores; W row-sharded to match; partial
  features all-reduced (tiny [256,512]) before the loss tail.
- The compressed view (keep k<T/2) has an EXACTLY sparse circulant kernel
  (delta/2 + 1/T on odd lags), so its feature is derived post-all-reduce as
  0.5*f0 + a parity-sum correction (host-side from x and W column parity
  sums, summed across cores) -- no circulant at all.
- The distorted/combined circulant kernels decay ~1/n; banded to the single
  block-diagonal (128-wide) of the circulant (final-loss rel err 8.3e-5
  measured host-side against the exact pipeline; gate is 2e-2).
- s2 (noise scale of the combined view) is computed host-side via FFT, so
  the kernel is ONE fused loop over t-blocks: banded circulants for
  dist/comb + packed feature matmuls, no separate stats pass.
- bf16 upstream of the feature all-reduce; noise tensors in fp8-e4m3 (their
  feature contribution is NOISE_STD-scaled, so fp8 error is negligible --
  measured 8.3e-5 host-side). W pre-transposed host-side to [T, CH*D] so
  large contiguous DMAs feed 8 channels at a time.
- Small constants are packed into two tensors (cb bf16 / cf f32): per-exec
  runtime overhead on this pool scales with the number of input bindings.
- noisy / combined use linearity: f(x + s*n) = f(x) + f(s*n); feature
  matmuls pack 2 view-groups of 64 rows per 128-row matmul:
  (x | s1*n1) and (dist | zt + s2*n2), accumulating over (tb, cl) against
  each streamed W chunk (W read once, in bf16).
- N_CORES=2: fastest end-to-end on this pool -- per-exec dispatch overhead
  grows ~0.2 ms/core while per-core device time shrinks sublinearly.
"""
import numpy as np

B, C, T, D = 64, 64, 4096, 512
N_CORES = 2
TBS = T // 128               # 32 t blocks
NOISE_STD = 0.02
TEMP = 0.1
NV = 5                       # views
AR_ROWS = 4 * 64             # x, s1n1, dist, comb partials
CF_ROWS = 64 + 1 + NV * B + 128   # corr | bias | maskmat+cnt | tmaskc
CB_ROWS = 2 + 2 * 128             # s1 | s2 | kern

_NC_CACHE = {}


def _host_consts(freq_start, time_start):
    k = np.arange(T)
    keep3072 = (k < int(T * 0.75)).astype(np.float64)
    fw = int(0.1 * T)
    fmask = np.where((k >= freq_start) & (k < freq_start + fw), 0.1, 1.0)
    tw = int(0.05 * T)
    tmask = np.where((k >= time_start) & (k < time_start + tw), 0.1, 1.0)
    m1s = (keep3072 + keep3072[(-k) % T]) / 2.0

    cs = [np.real(np.fft.ifft(m)) for m in (fmask, m1s * fmask)]

    # single block-diagonal circulant lhsT tiles: kern[v, j, i] = c_v[(i-j)%T]
    jj = np.arange(128)[:, None]
    ii = np.arange(128)[None, :]
    idx = (ii - jj) % T
    kern = np.stack([c[idx] for c in cs])          # [2, 128, 128]

    # t-mask per-partition columns for every affected tb
    tb_aff = sorted({t // 128 for t in range(time_start, time_start + tw)})
    tcols = np.stack([tmask[tb * 128:(tb + 1) * 128] for tb in tb_aff],
                     axis=1).astype(np.float32)    # [128, n_aff]

    n = NV * B
    maskmat = (np.eye(n, k=1) + np.eye(n, k=-1)).astype(np.float32)
    cnt = maskmat.sum(1, keepdims=True).astype(np.float32)   # [320, 1]

    # symmetrized spectral mask for host-side s2 (rfft half-spectrum)
    fmask_s = (fmask + fmask[(-k) % T]) / 2.0
    msym_half = (m1s * fmask_s)[:T // 2 + 1]
    return kern, tb_aff, tcols, maskmat, cnt, tmask, msym_half


def _build_nc(tb_aff, n_cores, use_collective):
    import concourse.bacc as bacc
    import concourse.mybir as mybir
    import concourse.tile as tile
    from concourse.masks import make_identity

    DT = mybir.dt.float32
    BF = mybir.dt.bfloat16
    F8 = mybir.dt.float8e4
    F32R = mybir.dt.float32r
    AF = mybir.ActivationFunctionType
    n_aff = len(tb_aff)
    CH = C // n_cores        # channels per core
    R = B * CH               # feature rows per core
    G = max(CH // 8, 1)      # W DMA channel groups
    GC = CH // G             # channels per group
    RC = max(R // 512, 1)    # circulant psum chunks
    RW = R // RC

    nc = bacc.Bacc("TRN2", target_bir_lowering=False, debug=False,
                   num_devices=n_cores)

    x_d = nc.dram_tensor("xts", [T, R], BF, kind="ExternalInput").ap()
    n12_d = nc.dram_tensor("n12", [T, 2 * R], F8, kind="ExternalInput").ap()
    w_d = nc.dram_tensor("Ws", [T, CH * D], BF, kind="ExternalInput").ap()
    cb_d = nc.dram_tensor("cb", [CB_ROWS, R], BF, kind="ExternalInput").ap()
    cf_d = nc.dram_tensor("cf", [CF_ROWS, 512], DT, kind="ExternalInput").ap()
    out_d = nc.dram_tensor("out_loss", [1, 1], DT, kind="ExternalOutput").ap()

    ar_in = nc.dram_tensor("ar_in", [AR_ROWS, D], DT).ap()
    # collective shared-output addressing is only supported for >4 cores
    ar_kw = {"addr_space": "Shared"} if n_cores > 4 else {}
    ar_out = nc.dram_tensor("ar_out", [AR_ROWS, D], DT, **ar_kw).ap()

    # cf row map
    CORR0, BIAS0, MM0, TMC0 = 0, 64, 65, 65 + NV * B

    def mmb(out, lhsT, rhs, start, stop):
        nc.tensor.matmul(out, lhsT, rhs, start=start, stop=stop)

    with tile.TileContext(nc) as tc:
      with tc.tile_pool(name="const", bufs=1) as cp:
        kern_sb = cp.tile([128, 2 * 128], BF, tag="kern")
        nc.sync.dma_start(
            kern_sb[:].rearrange("j (g i) -> j g i", i=128),
            cb_d[2:2 + 256, 0:128].rearrange("(g j) i -> j g i", g=2))
        ident = cp.tile([128, 128], DT, tag="ident")
        make_identity(nc, ident[:])
        ones_raw = cp.tile([128, 1], DT, tag="ones_raw")
        nc.vector.memset(ones_raw[:], 1.0)
        ones = cp.tile([128, 1], DT, tag="ones")
        nc.scalar.copy(ones[:].bitcast(F32R), ones_raw[:])
        tmc = cp.tile([128, n_aff], DT, tag="tmc")
        nc.sync.dma_start(tmc[:], cf_d[TMC0:TMC0 + 128, 0:n_aff])
        s1b = cp.tile([128, R], BF, tag="s1b")
        s2b = cp.tile([128, R], BF, tag="s2b")
        corr_sb = cp.tile([64, D], DT, tag="corr")

        def kslice(v):
            return kern_sb[:, v * 128:(v + 1) * 128]

        with tc.tile_pool(name="fps", bufs=1, space="PSUM") as fps:
            nc.gpsimd.dma_start(out=s1b[:],
                                in_=cb_d[0:1, :].to_broadcast((128, R)))
            nc.gpsimd.dma_start(out=s2b[:],
                                in_=cb_d[1:2, :].to_broadcast((128, R)))
            nc.sync.dma_start(corr_sb[:], cf_d[CORR0:CORR0 + 64, :])

            # ---------- fused main loop: circulants + features ----------
            f1_ps = fps.tile([128, D], DT, tag="f1")   # x | s1*n1
            f2_ps = fps.tile([128, D], DT, tag="f2")   # dist | zt + s2*n2
            with (
                tc.tile_pool(name="pc_sb", bufs=1) as pc,
                tc.tile_pool(name="pc_ps", bufs=1, space="PSUM") as pcps,
            ):
                for tb in range(TBS):
                    rows = slice(tb * 128, (tb + 1) * 128)
                    px = pc.tile([128, 2 * R], BF, tag="px", bufs=2)
                    nc.sync.dma_start(px[:, 0:R], x_d[rows, :])
                    nf8 = pc.tile([128, 2 * R], F8, tag="nf8", bufs=2)
                    nc.sync.dma_start(nf8[:], n12_d[rows, :])
                    nc.scalar.copy(px[:, R:2 * R], nf8[:, 0:R])
                    nc.vector.tensor_mul(px[:, R:2 * R], px[:, R:2 * R],
                                         s1b[:])
                    a = tb_aff.index(tb) if tb in tb_aff else None
                    # combined-view banded circulant (chunked over R)
                    ztv = pc.tile([128, R], BF, tag="ztv", bufs=2)
                    for rc in range(RC):
                        sl = slice(rc * RW, (rc + 1) * RW)
                        zc = pcps.tile([128, RW], DT, tag="zc", bufs=2)
                        mmb(zc[:], kslice(1), px[:, sl], True, True)
                        if a is not None:
                            nc.vector.tensor_scalar_mul(ztv[:, sl], zc[:],
                                                        tmc[:, a:a + 1])
                        else:
                            nc.scalar.copy(ztv[:, sl], zc[:])
                    pz = pc.tile([128, 2 * R], BF, tag="pz", bufs=2)
                    nc.scalar.copy(pz[:, R:2 * R], nf8[:, R:2 * R])
                    nc.vector.tensor_mul(pz[:, R:2 * R], pz[:, R:2 * R],
                                         s2b[:])
                    nc.vector.tensor_add(pz[:, R:2 * R], pz[:, R:2 * R],
                                         ztv[:])
                    # distorted-view banded circulant (chunked over R)
                    for rc in range(RC):
                        sl = slice(rc * RW, (rc + 1) * RW)
                        zd = pcps.tile([128, RW], DT, tag="zd", bufs=2)
                        mmb(zd[:], kslice(0), px[:, sl], True, True)
                        if a is not None:
                            nc.vector.tensor_scalar_mul(pz[:, sl], zd[:],
                                                        tmc[:, a:a + 1])
                        else:
                            nc.scalar.copy(pz[:, sl], zd[:])
                    # feature matmuls against streamed W chunks
                    for g in range(G):
                        wb = pc.tile([128, GC * D], BF, tag="w", bufs=3)
                        nc.sync.dma_start(
                            wb[:],
                            w_d[rows, g * GC * D:(g + 1) * GC * D])
                        for c8 in range(GC):
                            cl = g * GC + c8
                            st = tb == 0 and cl == 0
                            sp = tb == TBS - 1 and cl == CH - 1
                            wsl = wb[:, c8 * D:(c8 + 1) * D]
                            for pt, fp in ((px, f1_ps), (pz, f2_ps)):
                                lhs = pt[:].rearrange(
                                    "p (v b c) -> p v b c",
                                    v=2, c=CH)[:, :, :, cl]
                                mmb(fp[:], lhs, wsl, st, sp)

            # ---------- all-reduce partial features ----------
            with tc.tile_pool(name="pd_sb", bufs=1) as pd:
                fsb = [pd.tile([128, D], DT, tag=f"fsb{i}", name=f"fsb{i}")
                       for i in range(2)]
                nc.scalar.copy(fsb[0][:], f1_ps[:])
                nc.scalar.copy(fsb[1][:], f2_ps[:])
                # ar rows: 0:64 x, 64:128 s1n1, 128:192 dist, 192:256 comb
                nc.gpsimd.dma_start(ar_in[0:128], fsb[0][:])
                nc.gpsimd.dma_start(ar_in[128:256], fsb[1][:])
                if use_collective:
                    nc.gpsimd.collective_compute(
                        "AllReduce", mybir.AluOpType.add,
                        replica_groups=[list(range(n_cores))],
                        ins=[ar_in], outs=[ar_out])
                else:
                    nc.gpsimd.dma_start(ar_out, ar_in)

        # ---------- loss tail (identical on every core) ----------
        with (
            tc.tile_pool(name="pe_sb", bufs=1) as pe,
            tc.tile_pool(name="pe_ps", bufs=1, space="PSUM") as peps,
        ):
            bb = pe.tile([128, D], DT, tag="bb")
            nc.gpsimd.dma_start(
                out=bb[:], in_=cf_d[BIAS0:BIAS0 + 1, :].to_broadcast((128, D)))
            ag = [pe.tile([64, D], DT, tag=f"ag{v}", name=f"ag{v}")
                  for v in range(4)]
            for v in range(4):
                nc.sync.dma_start(ag[v][:], ar_out[v * 64:(v + 1) * 64])
            # features (dict order): x, compressed, distorted, noisy, combined
            fv = [pe.tile([64, D], DT, tag=f"fv{v}", name=f"fv{v}")
                  for v in range(NV)]
            nc.vector.tensor_add(fv[0][:], ag[0][:], bb[0:64, :])      # x
            nc.vector.tensor_scalar_mul(fv[1][:], ag[0][:], 0.5)       # comp
            nc.vector.tensor_add(fv[1][:], fv[1][:], corr_sb[:])
            nc.vector.tensor_add(fv[1][:], fv[1][:], bb[0:64, :])
            nc.vector.tensor_add(fv[2][:], ag[2][:], bb[0:64, :])      # dist
            nc.vector.tensor_add(fv[3][:], ag[0][:], ag[1][:])         # noisy
            nc.vector.tensor_add(fv[3][:], fv[3][:], bb[0:64, :])
            nc.vector.tensor_add(fv[4][:], ag[3][:], bb[0:64, :])      # comb

            # consistency: sum over v of ||f0 - fv||^2
            cacc = pe.tile([64, 4], DT, tag="cacc")
            for v in range(1, NV):
                dd = pe.tile([64, D], DT, tag="dd", bufs=2)
                nc.vector.tensor_sub(dd[:], fv[v][:], fv[0][:])
                dsq = pe.tile([64, D], DT, tag="dsq", bufs=2)
                nc.scalar.activation(dsq[:], dd[:], AF.Square,
                                     accum_out=cacc[:, v - 1:v])
            cps = peps.tile([1, 4], DT, tag="smallps")
            nc.tensor.matmul(cps[:], ones[0:64, :], cacc[:],
                             start=True, stop=True)
            csb = pe.tile([1, 4], DT, tag="csb")
            nc.scalar.copy(csb[:], cps[:])
            cons = pe.tile([1, 1], DT, tag="cons")
            nc.vector.tensor_reduce(cons[:], csb[:], mybir.AxisListType.X,
                                    mybir.AluOpType.add)

            # normalize rows
            for v in range(NV):
                nrm = pe.tile([64, 1], DT, tag="nrm", bufs=2)
                scr = pe.tile([64, D], DT, tag="scr", bufs=2)
                nc.scalar.activation(scr[:], fv[v][:], AF.Square,
                                     accum_out=nrm[:])
                nc.scalar.sqrt(nrm[:], nrm[:])
                rnr = pe.tile([64, 1], DT, tag="rnr", bufs=2)
                nc.vector.reciprocal(rnr[:], nrm[:])
                nc.vector.tensor_scalar_mul(fv[v][:], fv[v][:], rnr[:])

            # fnT [d-part, 320]
            fnT = [pe.tile([128, NV * B], DT, tag=f"fnT{dc}", name=f"fnT{dc}")
                   for dc in range(4)]
            for v in range(NV):
                for dc in range(4):
                    tp = peps.tile([128, 64], DT, tag="ttr", bufs=2)
                    nc.tensor.transpose(
                        tp[:], fv[v][:, dc * 128:(dc + 1) * 128],
                        ident[0:64, 0:64])
                    nc.scalar.copy(fnT[dc][:, v * 64:(v + 1) * 64], tp[:])

            # sim rows, logsumexp (no max shift: |sim/T|<=10), masked sums
            mrow = [0, 128, 256]
            mlen = [128, 128, 64]
            parts = []
            for rk in range(3):
                n_r = mlen[rk]
                sps = peps.tile([n_r, NV * B], DT, tag="sps", bufs=2)
                for dc in range(4):
                    lhs = fnT[dc][:, mrow[rk]:mrow[rk] + n_r]
                    nc.tensor.matmul(sps[:], lhs, fnT[dc][:],
                                     start=dc == 0, stop=dc == 3)
                sim = pe.tile([n_r, NV * B], DT, tag=f"sim{rk}")
                nc.scalar.copy(sim[:], sps[:])
                esc = pe.tile([n_r, NV * B], DT, tag="esc", bufs=2)
                sume = pe.tile([n_r, 1], DT, tag="sume", bufs=2)
                nc.scalar.activation(esc[:], sim[:], AF.Exp,
                                     scale=10.0, accum_out=sume[:])
                lse = pe.tile([n_r, 1], DT, tag="lse", bufs=2)
                nc.scalar.activation(lse[:], sume[:], AF.Ln)
                # masked raw sum
                mmt = pe.tile([n_r, NV * B], DT, tag="mmt", bufs=2)
                nc.sync.dma_start(
                    mmt[:], cf_d[MM0 + mrow[rk]:MM0 + mrow[rk] + n_r, 0:320])
                nc.vector.tensor_mul(mmt[:], mmt[:], sim[:])
                mr = pe.tile([n_r, 1], DT, tag="mr", bufs=2)
                nc.vector.tensor_reduce(mr[:], mmt[:], mybir.AxisListType.X,
                                        mybir.AluOpType.add)
                nc.vector.tensor_scalar_mul(mr[:], mr[:], 10.0)
                cntt = pe.tile([n_r, 1], DT, tag="cntt", bufs=2)
                nc.sync.dma_start(
                    cntt[:],
                    cf_d[MM0 + mrow[rk]:MM0 + mrow[rk] + n_r, 320:321])
                nc.vector.tensor_mul(cntt[:], cntt[:], lse[:])
                nc.vector.tensor_sub(mr[:], mr[:], cntt[:])
                parts.append(mr)
            stk = pe.tile([128, 3], DT, tag="stk")
            nc.vector.memset(stk[:], 0.0)
            nc.scalar.copy(stk[:, 0:1], parts[0][:])
            nc.scalar.copy(stk[:, 1:2], parts[1][:])
            nc.scalar.copy(stk[0:64, 2:3], parts[2][:])
            mps = peps.tile([1, 3], DT, tag="smallps")
            nc.tensor.matmul(mps[:], ones[:], stk[:], start=True, stop=True)
            msb = pe.tile([1, 3], DT, tag="msb")
            nc.scalar.copy(msb[:], mps[:])
            msum = pe.tile([1, 1], DT, tag="msum")
            nc.vector.tensor_reduce(msum[:], msb[:], mybir.AxisListType.X,
                                    mybir.AluOpType.add)

            # total = cons/(4*B*D) - 0.5 * msum / (2*NV*B - 2)
            nc.scalar.mul(cons[:], cons[:], 1.0 / (4 * B * D))
            nc.scalar.mul(msum[:], msum[:], -0.5 / float(2 * NV * B - 2))
            tot = pe.tile([1, 1], DT, tag="tot")
            nc.vector.tensor_add(tot[:], cons[:], msum[:])
            nc.sync.dma_start(out_d, tot[:])

    nc.compile()
    return nc


def _get_nc(tb_aff, n_cores, use_collective):
    key = (tuple(tb_aff), n_cores, use_collective)
    if key not in _NC_CACHE:
        _NC_CACHE[key] = _build_nc(list(tb_aff), n_cores, use_collective)
    return _NC_CACHE[key]


def make_in_maps(x, W, b, noise1, noise2, freq_start, time_start,
                 n_cores=None):
    import ml_dtypes
    BF16 = ml_dtypes.bfloat16
    F8 = ml_dtypes.float8_e4m3
    if n_cores is None:
        n_cores = N_CORES
    CH = C // n_cores
    R = B * CH
    kern, tb_aff, tcols, maskmat, cnt, tmask, msym_half = _host_consts(
        int(freq_start), int(time_start))
    n_aff = len(tb_aff)
    x = np.asarray(x, dtype=np.float32)
    W = np.asarray(W, dtype=np.float32)
    b = np.asarray(b, dtype=np.float32)
    xbf = x.astype(BF16)
    n1f8 = np.asarray(noise1, dtype=np.float32).astype(F8)
    n2f8 = np.asarray(noise2, dtype=np.float32).astype(F8)

    # host-side noise scales: s1 from x, s2 from the exact combined view
    z = np.fft.irfft(np.fft.rfft(x, axis=-1) * msym_half, axis=-1)
    zt = z * tmask
    s1_all = (NOISE_STD * x.std(-1, ddof=1)).astype(BF16)       # [B, C]
    s2_all = (NOISE_STD * zt.std(-1, ddof=1)).astype(BF16)      # [B, C]

    # exact compressed-view correction, summed over all channels (fp32)
    Wr = W.reshape(C, T, D)
    WeT = Wr[:, 0::2, :].sum(axis=1) * (1.0 / T)   # [C, D]
    WoT = Wr[:, 1::2, :].sum(axis=1) * (1.0 / T)
    s_e = x[:, :, 0::2].sum(-1)                    # [B, C]
    s_o = x[:, :, 1::2].sum(-1)
    corr_tot = (s_o @ WeT + s_e @ WoT).astype(np.float32)       # [B, D]

    # packed f32 consts: corr | bias | maskmat+cnt | tmaskc
    cf = np.zeros((CF_ROWS, 512), np.float32)
    cf[0:64] = corr_tot
    cf[64, :] = b
    cf[65:65 + NV * B, 0:NV * B] = maskmat
    cf[65:65 + NV * B, NV * B:NV * B + 1] = cnt
    cf[65 + NV * B:65 + NV * B + 128, 0:n_aff] = tcols

    Wbf = W.astype(BF16).reshape(C, T, D)
    kern_bf = kern.astype(BF16)
    in_maps = []
    for core in range(n_cores):
        cs = core * CH
        # packed bf16 consts: s1 | s2 | kern
        cb = np.zeros((CB_ROWS, R), BF16)
        cb[0, :] = s1_all[:, cs:cs + CH].reshape(R)
        cb[1, :] = s2_all[:, cs:cs + CH].reshape(R)
        cb[2:2 + 256, 0:128] = kern_bf.reshape(256, 128)
        n1ts = n1f8[:, cs:cs + CH, :].reshape(R, T).T
        n2ts = n2f8[:, cs:cs + CH, :].reshape(R, T).T
        in_maps.append({
            "xts": np.ascontiguousarray(
                xbf[:, cs:cs + CH, :].reshape(R, T).T),
            "n12": np.ascontiguousarray(
                np.concatenate([n1ts, n2ts], axis=1)),
            "Ws": np.ascontiguousarray(
                Wbf[cs:cs + CH].transpose(1, 0, 2).reshape(T, CH * D)),
            "cb": cb,
            "cf": cf,
        })
    return in_maps, tb_aff


def kernel(x, W, b, noise1, noise2, freq_start, time_start):
    from concourse.bass_utils import run_bass_kernel_spmd
    in_maps, tb_aff = make_in_maps(x, W, b, noise1, noise2,
                                   freq_start, time_start)
    nc = _get_nc(tb_aff, N_CORES, N_CORES > 1)
    res = run_bass_kernel_spmd(nc, in_maps, core_ids=list(range(N_CORES)))
    return np.float32(res.results[0]["out_loss"].reshape(())[()])


# revision 12
# speedup vs baseline: 3.0800x; 1.1059x over previous
"""Trainium2 Bass kernel for nn_CompressionAugmentedTrainer.

Strategy (SPMD, channel-sharded across N_CORES cores):
- Shard C=64 channels across cores; W row-sharded to match; partial
  features all-reduced (tiny [256,512]) before the loss tail.
- The compressed view (keep k<T/2) has an EXACTLY sparse circulant kernel
  (delta/2 + 1/T on odd lags), so its feature is derived post-all-reduce as
  0.5*f0 + a parity-sum correction (host-side from x and W column parity
  sums, summed across cores) -- no circulant at all.
- The distorted/combined circulant kernels decay ~1/n; banded to the single
  block-diagonal (128-wide) of the circulant.
- s2 (noise scale of the combined view) is computed host-side via FFT, so
  the kernel is ONE fused loop over t-block pairs: banded circulants for
  dist/comb + packed feature matmuls, no separate stats pass.
- The whole feature path runs in fp8-e4m3 with fp32 PSUM accumulation and
  DoubleRow perf-mode matmuls (2 t-blocks contracted per instruction at 2x
  PE throughput, half the W DMA bytes). W-quantization error is common-mode
  between f0 and the augmented views, so it largely cancels in the loss:
  measured final rel err 6.8e-4 host-side vs the exact pipeline (gate 2e-2).
- Host packs x|n1|n2 pair-interleaved into one fp8 tensor (one DMA per
  block pair) and W as [T/2, CH*2*D] (c, i, d) so each channel's rhs slice
  is DoubleRow-ready; small constants are packed into two tensors (per-exec
  runtime overhead on this pool scales with the number of input bindings).
- noisy / combined use linearity: f(x + s*n) = f(x) + f(s*n); feature
  matmuls pack 2 view-groups of 64 rows per 128-row matmul:
  (x | s1*n1) and (dist | zt + s2*n2), accumulating over (tb2, cl).
- N_CORES=2: fastest end-to-end on this pool -- per-exec dispatch overhead
  grows ~0.2 ms/core while per-core device time shrinks sublinearly.
"""
import numpy as np

B, C, T, D = 64, 64, 4096, 512
N_CORES = 2
TBS = T // 128               # 32 t blocks
TP = TBS // 2                # 16 t-block pairs
NOISE_STD = 0.02
TEMP = 0.1
NV = 5                       # views
AR_ROWS = 4 * 64             # x, s1n1, dist, comb partials
CF_ROWS = 64 + 1 + NV * B + 128   # corr | bias | maskmat+cnt | tmaskc

_NC_CACHE = {}


def _host_consts(freq_start, time_start):
    k = np.arange(T)
    keep3072 = (k < int(T * 0.75)).astype(np.float64)
    fw = int(0.1 * T)
    fmask = np.where((k >= freq_start) & (k < freq_start + fw), 0.1, 1.0)
    tw = int(0.05 * T)
    tmask = np.where((k >= time_start) & (k < time_start + tw), 0.1, 1.0)
    m1s = (keep3072 + keep3072[(-k) % T]) / 2.0

    cs = [np.real(np.fft.ifft(m)) for m in (fmask, m1s * fmask)]

    # single block-diagonal circulant lhsT tiles: kern[v, j, i] = c_v[(i-j)%T]
    jj = np.arange(128)[:, None]
    ii = np.arange(128)[None, :]
    idx = (ii - jj) % T
    kern = np.stack([c[idx] for c in cs])          # [2, 128, 128]

    # t-mask per-partition columns for every affected tb
    tb_aff = sorted({t // 128 for t in range(time_start, time_start + tw)})
    tcols = np.stack([tmask[tb * 128:(tb + 1) * 128] for tb in tb_aff],
                     axis=1).astype(np.float32)    # [128, n_aff]

    n = NV * B
    maskmat = (np.eye(n, k=1) + np.eye(n, k=-1)).astype(np.float32)
    cnt = maskmat.sum(1, keepdims=True).astype(np.float32)   # [320, 1]

    # symmetrized spectral mask for host-side s2 (rfft half-spectrum)
    fmask_s = (fmask + fmask[(-k) % T]) / 2.0
    msym_half = (m1s * fmask_s)[:T // 2 + 1]
    return kern, tb_aff, tcols, maskmat, cnt, tmask, msym_half


def _build_nc(tb_aff, n_cores, use_collective):
    import concourse.bacc as bacc
    import concourse.mybir as mybir
    import concourse.tile as tile
    from concourse.masks import make_identity

    DT = mybir.dt.float32
    BF = mybir.dt.bfloat16
    F8 = mybir.dt.float8e4
    F32R = mybir.dt.float32r
    AF = mybir.ActivationFunctionType
    DR = mybir.MatmulPerfMode.DoubleRow
    n_aff = len(tb_aff)
    CH = C // n_cores        # channels per core
    R = B * CH               # feature rows per core
    G = max(CH // 8, 1)      # W DMA channel groups
    GC = CH // G             # channels per group
    RC = max(R // 512, 1)    # circulant psum chunks
    RW = R // RC

    nc = bacc.Bacc("TRN2", target_bir_lowering=False, debug=False,
                   num_devices=n_cores)

    # x | s1*n1 | s2*n2 pair-interleaved: row tp*128+p, col i*3R + w*R + r
    # holds tensor w at t=(2*tp+i)*128+p, feature-row r (noise pre-scaled
    # host-side); kern appended as rows T/2 .. T/2+256 (cols 0:128).
    xk_d = nc.dram_tensor("xn12k", [T // 2 + 256, 6 * R], F8,
                          kind="ExternalInput").ap()
    # W pair-interleaved: row tp*128+p, col (c*2 + i)*D + d.
    w_d = nc.dram_tensor("Ws", [T // 2, CH * 2 * D], F8,
                         kind="ExternalInput").ap()
    cf_d = nc.dram_tensor("cf", [CF_ROWS, 512], DT, kind="ExternalInput").ap()
    out_d = nc.dram_tensor("out_loss", [1, 1], DT, kind="ExternalOutput").ap()

    ar_in = nc.dram_tensor("ar_in", [AR_ROWS, D], DT).ap()
    # collective shared-output addressing is only supported for >4 cores
    ar_kw = {"addr_space": "Shared"} if n_cores > 4 else {}
    ar_out = nc.dram_tensor("ar_out", [AR_ROWS, D], DT, **ar_kw).ap()

    # cf row map
    CORR0, BIAS0, MM0, TMC0 = 0, 64, 65, 65 + NV * B

    def mmb(out, lhsT, rhs, start, stop, perf_mode=None):
        nc.tensor.matmul(out, lhsT, rhs, start=start, stop=stop,
                         perf_mode=perf_mode)

    with tile.TileContext(nc) as tc:
      with tc.tile_pool(name="const", bufs=1) as cp:
        kern_sb = cp.tile([128, 2 * 128], F8, tag="kern")
        nc.sync.dma_start(
            kern_sb[:].rearrange("j (g i) -> j g i", i=128),
            xk_d[T // 2:T // 2 + 256, 0:128].rearrange(
                "(g j) i -> j g i", g=2))
        ident = cp.tile([128, 128], DT, tag="ident")
        make_identity(nc, ident[:])
        ident8 = cp.tile([128, 128], F8, tag="ident8")
        nc.scalar.copy(ident8[:], ident[:])
        ones_raw = cp.tile([128, 1], DT, tag="ones_raw")
        nc.vector.memset(ones_raw[:], 1.0)
        ones = cp.tile([128, 1], DT, tag="ones")
        nc.scalar.copy(ones[:].bitcast(F32R), ones_raw[:])
        tmc = cp.tile([128, n_aff], DT, tag="tmc")
        nc.sync.dma_start(tmc[:], cf_d[TMC0:TMC0 + 128, 0:n_aff])
        corr_sb = cp.tile([64, D], DT, tag="corr")

        def kslice(v):
            return kern_sb[:, v * 128:(v + 1) * 128]

        with tc.tile_pool(name="fps", bufs=1, space="PSUM") as fps:
            nc.sync.dma_start(corr_sb[:], cf_d[CORR0:CORR0 + 64, :])

            # ---------- fused main loop over t-block pairs ----------
            f1_ps = fps.tile([128, D], DT, tag="f1")   # x | s1*n1
            f2_ps = fps.tile([128, D], DT, tag="f2")   # dist | zt + s2*n2
            with (
                tc.tile_pool(name="pc_sb", bufs=1) as pc,
                tc.tile_pool(name="pc_ps", bufs=1, space="PSUM") as pcps,
            ):
                for tp in range(TP):
                    rows = slice(tp * 128, (tp + 1) * 128)
                    # nf: [128, (i, w=x|s1n1|s2n2, b, c)] fp8, one DMA
                    nf = pc.tile([128, 6 * R], F8, tag="nf", bufs=2)
                    nc.sync.dma_start(nf[:], xk_d[rows, :])
                    # pz2: [128, (i, v=dist|comb, b, c)] fp8
                    pz2 = pc.tile([128, 4 * R], F8, tag="pz", bufs=2)
                    for i in range(2):
                        tb = 2 * tp + i
                        a = tb_aff.index(tb) if tb in tb_aff else None
                        xsl = nf[:, i * 3 * R:i * 3 * R + R]
                        n2sl = nf[:, i * 3 * R + 2 * R:i * 3 * R + 3 * R]
                        cdst = pz2[:, (2 * i + 1) * R:(2 * i + 2) * R]
                        for rc in range(RC):
                            sl = slice(rc * RW, (rc + 1) * RW)
                            # combined view: circ(x) (+ s2*n2 in psum on the
                            # unmasked fast path), then -> fp8
                            zc = pcps.tile([128, RW], DT, tag="zc", bufs=2)
                            if a is None:
                                mmb(zc[:], kslice(1), xsl[:, sl], True, False)
                                mmb(zc[:], ident8[:], n2sl[:, sl],
                                    False, True)
                                nc.vector.tensor_scalar_mul(
                                    cdst[:, sl], zc[:], 1.0)
                            else:
                                mmb(zc[:], kslice(1), xsl[:, sl], True, True)
                                ztv = pc.tile([128, RW], BF, tag="ztv",
                                              bufs=2)
                                nc.vector.tensor_scalar_mul(
                                    ztv[:], zc[:], tmc[:, a:a + 1])
                                n2b = pc.tile([128, RW], BF, tag="n2b",
                                              bufs=2)
                                nc.scalar.copy(n2b[:], n2sl[:, sl])
                                nc.vector.tensor_add(ztv[:], ztv[:], n2b[:])
                                nc.scalar.copy(cdst[:, sl], ztv[:])
                            # distorted view -> fp8 (i, v=0)
                            zd = pcps.tile([128, RW], DT, tag="zd", bufs=2)
                            mmb(zd[:], kslice(0), xsl[:, sl], True, True)
                            dst = pz2[:, 2 * i * R + rc * RW:
                                      2 * i * R + (rc + 1) * RW]
                            if a is not None:
                                nc.vector.tensor_scalar_mul(
                                    dst, zd[:], tmc[:, a:a + 1])
                            else:
                                nc.scalar.copy(dst, zd[:])
                    # feature matmuls: DoubleRow over the (i) pair
                    for g in range(G):
                        wb = pc.tile([128, GC * 2 * D], F8, tag="w", bufs=3)
                        nc.sync.dma_start(
                            wb[:],
                            w_d[rows, g * GC * 2 * D:(g + 1) * GC * 2 * D])
                        for c8 in range(GC):
                            cl = g * GC + c8
                            st = tp == 0 and cl == 0
                            sp = tp == TP - 1 and cl == CH - 1
                            # rhs: [128, (i, d)] for channel cl
                            wsl = wb[:].rearrange(
                                "p (cc i d) -> p cc i d",
                                cc=GC, i=2)[:, c8]
                            lhs1 = nf[:].rearrange(
                                "p (i w b c) -> p i w b c",
                                i=2, w=3, c=CH)[:, :, 0:2, :, cl]
                            lhs2 = pz2[:].rearrange(
                                "p (i v b c) -> p i v b c",
                                i=2, v=2, c=CH)[:, :, :, :, cl]
                            mmb(f1_ps[:], lhs1, wsl, st, sp, perf_mode=DR)
                            mmb(f2_ps[:], lhs2, wsl, st, sp, perf_mode=DR)

            # ---------- all-reduce partial features ----------
            with tc.tile_pool(name="pd_sb", bufs=1) as pd:
                fsb = [pd.tile([128, D], DT, tag=f"fsb{i}", name=f"fsb{i}")
                       for i in range(2)]
                nc.scalar.copy(fsb[0][:], f1_ps[:])
                nc.scalar.copy(fsb[1][:], f2_ps[:])
                # ar rows: 0:64 x, 64:128 s1n1, 128:192 dist, 192:256 comb
                nc.gpsimd.dma_start(ar_in[0:128], fsb[0][:])
                nc.gpsimd.dma_start(ar_in[128:256], fsb[1][:])
                if use_collective:
                    nc.gpsimd.collective_compute(
                        "AllReduce", mybir.AluOpType.add,
                        replica_groups=[list(range(n_cores))],
                        ins=[ar_in], outs=[ar_out])
                else:
                    nc.gpsimd.dma_start(ar_out, ar_in)

        # ---------- loss tail (identical on every core) ----------
        with (
            tc.tile_pool(name="pe_sb", bufs=1) as pe,
            tc.tile_pool(name="pe_ps", bufs=1, space="PSUM") as peps,
        ):
            bb = pe.tile([128, D], DT, tag="bb")
            nc.gpsimd.dma_start(
                out=bb[:], in_=cf_d[BIAS0:BIAS0 + 1, :].to_broadcast((128, D)))
            ag = [pe.tile([64, D], DT, tag=f"ag{v}", name=f"ag{v}")
                  for v in range(4)]
            for v in range(4):
                nc.sync.dma_start(ag[v][:], ar_out[v * 64:(v + 1) * 64])
            # features (dict order): x, compressed, distorted, noisy, combined
            fv = [pe.tile([64, D], DT, tag=f"fv{v}", name=f"fv{v}")
                  for v in range(NV)]
            nc.vector.tensor_add(fv[0][:], ag[0][:], bb[0:64, :])      # x
            nc.vector.tensor_scalar_mul(fv[1][:], ag[0][:], 0.5)       # comp
            nc.vector.tensor_add(fv[1][:], fv[1][:], corr_sb[:])
            nc.vector.tensor_add(fv[1][:], fv[1][:], bb[0:64, :])
            nc.vector.tensor_add(fv[2][:], ag[2][:], bb[0:64, :])      # dist
            nc.vector.tensor_add(fv[3][:], ag[0][:], ag[1][:])         # noisy
            nc.vector.tensor_add(fv[3][:], fv[3][:], bb[0:64, :])
            nc.vector.tensor_add(fv[4][:], ag[3][:], bb[0:64, :])      # comb

            # consistency: sum over v of ||f0 - fv||^2
            cacc = pe.tile([64, 4], DT, tag="cacc")
            for v in range(1, NV):
                dd = pe.tile([64, D], DT, tag="dd", bufs=2)
                nc.vector.tensor_sub(dd[:], fv[v][:], fv[0][:])
                dsq = pe.tile([64, D], DT, tag="dsq", bufs=2)
                nc.scalar.activation(dsq[:], dd[:], AF.Square,
                                     accum_out=cacc[:, v - 1:v])
            cps = peps.tile([1, 4], DT, tag="smallps")
            nc.tensor.matmul(cps[:], ones[0:64, :], cacc[:],
                             start=True, stop=True)
            csb = pe.tile([1, 4], DT, tag="csb")
            nc.scalar.copy(csb[:], cps[:])
            cons = pe.tile([1, 1], DT, tag="cons")
            nc.vector.tensor_reduce(cons[:], csb[:], mybir.AxisListType.X,
                                    mybir.AluOpType.add)

            # normalize rows
            for v in range(NV):
                nrm = pe.tile([64, 1], DT, tag="nrm", bufs=2)
                scr = pe.tile([64, D], DT, tag="scr", bufs=2)
                nc.scalar.activation(scr[:], fv[v][:], AF.Square,
                                     accum_out=nrm[:])
                nc.scalar.sqrt(nrm[:], nrm[:])
                rnr = pe.tile([64, 1], DT, tag="rnr", bufs=2)
                nc.vector.reciprocal(rnr[:], nrm[:])
                nc.vector.tensor_scalar_mul(fv[v][:], fv[v][:], rnr[:])

            # fnT [d-part, 320]
            fnT = [pe.tile([128, NV * B], DT, tag=f"fnT{dc}", name=f"fnT{dc}")
                   for dc in range(4)]
            for v in range(NV):
                for dc in range(4):
                    tp_ = peps.tile([128, 64], DT, tag="ttr", bufs=2)
                    nc.tensor.transpose(
                        tp_[:], fv[v][:, dc * 128:(dc + 1) * 128],
                        ident[0:64, 0:64])
                    nc.scalar.copy(fnT[dc][:, v * 64:(v + 1) * 64], tp_[:])

            # sim rows, logsumexp (no max shift: |sim/T|<=10), masked sums
            mrow = [0, 128, 256]
            mlen = [128, 128, 64]
            parts = []
            for rk in range(3):
                n_r = mlen[rk]
                sps = peps.tile([n_r, NV * B], DT, tag="sps", bufs=2)
                for dc in range(4):
                    lhs = fnT[dc][:, mrow[rk]:mrow[rk] + n_r]
                    nc.tensor.matmul(sps[:], lhs, fnT[dc][:],
                                     start=dc == 0, stop=dc == 3)
                sim = pe.tile([n_r, NV * B], DT, tag=f"sim{rk}")
                nc.scalar.copy(sim[:], sps[:])
                esc = pe.tile([n_r, NV * B], DT, tag="esc", bufs=2)
                sume = pe.tile([n_r, 1], DT, tag="sume", bufs=2)
                nc.scalar.activation(esc[:], sim[:], AF.Exp,
                                     scale=10.0, accum_out=sume[:])
                lse = pe.tile([n_r, 1], DT, tag="lse", bufs=2)
                nc.scalar.activation(lse[:], sume[:], AF.Ln)
                # masked raw sum
                mmt = pe.tile([n_r, NV * B], DT, tag="mmt", bufs=2)
                nc.sync.dma_start(
                    mmt[:], cf_d[MM0 + mrow[rk]:MM0 + mrow[rk] + n_r, 0:320])
                nc.vector.tensor_mul(mmt[:], mmt[:], sim[:])
                mr = pe.tile([n_r, 1], DT, tag="mr", bufs=2)
                nc.vector.tensor_reduce(mr[:], mmt[:], mybir.AxisListType.X,
                                        mybir.AluOpType.add)
                nc.vector.tensor_scalar_mul(mr[:], mr[:], 10.0)
                cntt = pe.tile([n_r, 1], DT, tag="cntt", bufs=2)
                nc.sync.dma_start(
                    cntt[:],
                    cf_d[MM0 + mrow[rk]:MM0 + mrow[rk] + n_r, 320:321])
                nc.vector.tensor_mul(cntt[:], cntt[:], lse[:])
                nc.vector.tensor_sub(mr[:], mr[:], cntt[:])
                parts.append(mr)
            stk = pe.tile([128, 3], DT, tag="stk")
            nc.vector.memset(stk[:], 0.0)
            nc.scalar.copy(stk[:, 0:1], parts[0][:])
            nc.scalar.copy(stk[:, 1:2], parts[1][:])
            nc.scalar.copy(stk[0:64, 2:3], parts[2][:])
            mps = peps.tile([1, 3], DT, tag="smallps")
            nc.tensor.matmul(mps[:], ones[:], stk[:], start=True, stop=True)
            msb = pe.tile([1, 3], DT, tag="msb")
            nc.scalar.copy(msb[:], mps[:])
            msum = pe.tile([1, 1], DT, tag="msum")
            nc.vector.tensor_reduce(msum[:], msb[:], mybir.AxisListType.X,
                                    mybir.AluOpType.add)

            # total = cons/(4*B*D) - 0.5 * msum / (2*NV*B - 2)
            nc.scalar.mul(cons[:], cons[:], 1.0 / (4 * B * D))
            nc.scalar.mul(msum[:], msum[:], -0.5 / float(2 * NV * B - 2))
            tot = pe.tile([1, 1], DT, tag="tot")
            nc.vector.tensor_add(tot[:], cons[:], msum[:])
            nc.sync.dma_start(out_d, tot[:])

    nc.compile()
    return nc


def _get_nc(tb_aff, n_cores, use_collective):
    key = (tuple(tb_aff), n_cores, use_collective)
    if key not in _NC_CACHE:
        _NC_CACHE[key] = _build_nc(list(tb_aff), n_cores, use_collective)
    return _NC_CACHE[key]


def make_in_maps(x, W, b, noise1, noise2, freq_start, time_start,
                 n_cores=None):
    import ml_dtypes
    BF16 = ml_dtypes.bfloat16
    F8 = ml_dtypes.float8_e4m3
    if n_cores is None:
        n_cores = N_CORES
    CH = C // n_cores
    R = B * CH
    kern, tb_aff, tcols, maskmat, cnt, tmask, msym_half = _host_consts(
        int(freq_start), int(time_start))
    n_aff = len(tb_aff)
    x = np.asarray(x, dtype=np.float32)
    W = np.asarray(W, dtype=np.float32)
    b = np.asarray(b, dtype=np.float32)
    xf8 = x.astype(F8)

    # host-side noise scales: s1 from x, s2 from the exact combined view;
    # noise shipped pre-scaled (f(x + s*n) needs s*n only)
    z = np.fft.irfft(np.fft.rfft(x, axis=-1) * msym_half, axis=-1)
    zt = z * tmask
    s1_all = (NOISE_STD * x.std(-1, ddof=1)).astype(np.float32)  # [B, C]
    s2_all = (NOISE_STD * zt.std(-1, ddof=1)).astype(np.float32)
    n1f8 = (np.asarray(noise1, dtype=np.float32)
            * s1_all[..., None]).astype(F8)
    n2f8 = (np.asarray(noise2, dtype=np.float32)
            * s2_all[..., None]).astype(F8)

    # exact compressed-view correction, summed over all channels (fp32)
    Wr = W.reshape(C, T, D)
    WeT = Wr[:, 0::2, :].sum(axis=1) * (1.0 / T)   # [C, D]
    WoT = Wr[:, 1::2, :].sum(axis=1) * (1.0 / T)
    s_e = x[:, :, 0::2].sum(-1)                    # [B, C]
    s_o = x[:, :, 1::2].sum(-1)
    corr_tot = (s_o @ WeT + s_e @ WoT).astype(np.float32)       # [B, D]

    # packed f32 consts: corr | bias | maskmat+cnt | tmaskc
    cf = np.zeros((CF_ROWS, 512), np.float32)
    cf[0:64] = corr_tot
    cf[64, :] = b
    cf[65:65 + NV * B, 0:NV * B] = maskmat
    cf[65:65 + NV * B, NV * B:NV * B + 1] = cnt
    cf[65 + NV * B:65 + NV * B + 128, 0:n_aff] = tcols

    Wf8 = W.astype(F8).reshape(C, T // 256, 2, 128, D)
    kern_f8 = kern.astype(F8)
    in_maps = []
    for core in range(n_cores):
        cs = core * CH
        # x|s1n1|s2n2 [T, 3R] -> pair-interleaved [T/2, 6R] + kern rows
        tri = np.concatenate(
            [t[:, cs:cs + CH, :].reshape(R, T).T
             for t in (xf8, n1f8, n2f8)], axis=1)          # [T, 3R]
        tri = tri.reshape(T // 256, 2, 128, 3 * R).transpose(0, 2, 1, 3) \
                 .reshape(T // 2, 6 * R)
        xk = np.zeros((T // 2 + 256, 6 * R), F8)
        xk[:T // 2] = tri
        xk[T // 2:, 0:128] = kern_f8.reshape(256, 128)
        # W [CH, T/256, 2, 128, D] -> [T/2, (c, i, d)]
        Wc = Wf8[cs:cs + CH].transpose(1, 3, 0, 2, 4) \
                            .reshape(T // 2, CH * 2 * D)
        in_maps.append({
            "xn12k": np.ascontiguousarray(xk),
            "Ws": np.ascontiguousarray(Wc),
            "cf": cf,
        })
    return in_maps, tb_aff


def kernel(x, W, b, noise1, noise2, freq_start, time_start):
    from concourse.bass_utils import run_bass_kernel_spmd
    in_maps, tb_aff = make_in_maps(x, W, b, noise1, noise2,
                                   freq_start, time_start)
    nc = _get_nc(tb_aff, N_CORES, N_CORES > 1)
    res = run_bass_kernel_spmd(nc, in_maps, core_ids=list(range(N_CORES)))
    return np.float32(res.results[0]["out_loss"].reshape(())[()])


# revision 13
# speedup vs baseline: 3.2918x; 1.0688x over previous
"""Trainium2 Bass kernel for nn_CompressionAugmentedTrainer.

Strategy (SPMD, channel-sharded across N_CORES cores):
- Shard C=64 channels across cores; W row-sharded to match; partial
  features all-reduced (tiny [256,512]) before the loss tail.
- The compressed view (keep k<T/2) has an EXACTLY sparse circulant kernel
  (delta/2 + 1/T on odd lags), so its feature is derived post-all-reduce as
  0.5*f0 + a parity-sum correction (host-side from x and W column parity
  sums, summed across cores) -- no circulant at all.
- The distorted/combined circulant kernels decay ~1/n; banded to the single
  block-diagonal (128-wide) of the circulant.
- s2 (noise scale of the combined view) is computed host-side via FFT, so
  the kernel is ONE fused loop over t-block pairs: banded circulants for
  dist/comb + packed feature matmuls, no separate stats pass.
- The whole feature path runs in fp8-e4m3 with fp32 PSUM accumulation and
  DoubleRow perf-mode matmuls (2 t-blocks contracted per instruction at 2x
  PE throughput, half the W DMA bytes). W-quantization error is common-mode
  between f0 and the augmented views, so it largely cancels in the loss:
  measured final rel err 6.8e-4 host-side vs the exact pipeline (gate 2e-2).
- Host packs x|n1|n2 pair-interleaved into one fp8 tensor (one DMA per
  block pair) and W as [T/2, CH*2*D] (c, i, d) so each channel's rhs slice
  is DoubleRow-ready; small constants are packed into two tensors (per-exec
  runtime overhead on this pool scales with the number of input bindings).
- noisy / combined use linearity: f(x + s*n) = f(x) + f(s*n); feature
  matmuls pack 2 view-groups of 64 rows per 128-row matmul:
  (x | s1*n1) and (dist | zt + s2*n2), accumulating over (tb2, cl).
- N_CORES=2: fastest end-to-end on this pool -- per-exec dispatch overhead
  grows ~0.2 ms/core while per-core device time shrinks sublinearly.
"""
import numpy as np

B, C, T, D = 64, 64, 4096, 512
N_CORES = 2
TBS = T // 128               # 32 t blocks
TP = TBS // 2                # 16 t-block pairs
NOISE_STD = 0.02
TEMP = 0.1
NV = 5                       # views
AR_ROWS = 4 * 64             # x, s1n1, dist, comb partials
CF_ROWS = 64 + 1 + NV * B + 128   # corr | bias | maskmat+cnt | tmaskc

_NC_CACHE = {}


def _host_consts(freq_start, time_start):
    k = np.arange(T)
    keep3072 = (k < int(T * 0.75)).astype(np.float64)
    fw = int(0.1 * T)
    fmask = np.where((k >= freq_start) & (k < freq_start + fw), 0.1, 1.0)
    tw = int(0.05 * T)
    tmask = np.where((k >= time_start) & (k < time_start + tw), 0.1, 1.0)
    m1s = (keep3072 + keep3072[(-k) % T]) / 2.0

    cs = [np.real(np.fft.ifft(m)) for m in (fmask, m1s * fmask)]

    # single block-diagonal circulant lhsT tiles: kern[v, j, i] = c_v[(i-j)%T]
    jj = np.arange(128)[:, None]
    ii = np.arange(128)[None, :]
    idx = (ii - jj) % T
    kern = np.stack([c[idx] for c in cs])          # [2, 128, 128]

    # t-mask per-partition columns for every affected tb
    tb_aff = sorted({t // 128 for t in range(time_start, time_start + tw)})
    tcols = np.stack([tmask[tb * 128:(tb + 1) * 128] for tb in tb_aff],
                     axis=1).astype(np.float32)    # [128, n_aff]

    n = NV * B
    maskmat = (np.eye(n, k=1) + np.eye(n, k=-1)).astype(np.float32)
    cnt = maskmat.sum(1, keepdims=True).astype(np.float32)   # [320, 1]

    # symmetrized spectral mask for host-side s2 (rfft half-spectrum)
    fmask_s = (fmask + fmask[(-k) % T]) / 2.0
    msym_half = (m1s * fmask_s)[:T // 2 + 1]
    return kern, tb_aff, tcols, maskmat, cnt, tmask, msym_half


def _build_nc(tb_aff, n_cores, use_collective):
    import concourse.bacc as bacc
    import concourse.mybir as mybir
    import concourse.tile as tile
    from concourse.masks import make_identity

    DT = mybir.dt.float32
    BF = mybir.dt.bfloat16
    F8 = mybir.dt.float8e4
    F32R = mybir.dt.float32r
    AF = mybir.ActivationFunctionType
    DR = mybir.MatmulPerfMode.DoubleRow
    n_aff = len(tb_aff)
    CH = C // n_cores        # channels per core
    R = B * CH               # feature rows per core
    G = max(CH // 8, 1)      # W DMA channel groups
    GC = CH // G             # channels per group
    RC = max(R // 512, 1)    # circulant psum chunks
    RW = R // RC

    nc = bacc.Bacc("TRN2", target_bir_lowering=False, debug=False,
                   num_devices=n_cores)

    # x | s1*n1 | s2*n2 pair-interleaved: row tp*128+p, col i*3R + w*R + r
    # holds tensor w at t=(2*tp+i)*128+p, feature-row r (noise pre-scaled
    # host-side); kern appended as rows T/2 .. T/2+256 (cols 0:128).
    xk_d = nc.dram_tensor("xn12k", [T // 2 + 256, 6 * R], F8,
                          kind="ExternalInput").ap()
    # W pair-interleaved: row tp*128+p, col (c*2 + i)*D + d.
    w_d = nc.dram_tensor("Ws", [T // 2, CH * 2 * D], F8,
                         kind="ExternalInput").ap()
    cf_d = nc.dram_tensor("cf", [CF_ROWS, 512], DT, kind="ExternalInput").ap()
    out_d = nc.dram_tensor("out_loss", [1, 1], DT, kind="ExternalOutput").ap()

    ar_in = nc.dram_tensor("ar_in", [AR_ROWS, D], DT).ap()
    # collective shared-output addressing is only supported for >4 cores
    ar_kw = {"addr_space": "Shared"} if n_cores > 4 else {}
    ar_out = nc.dram_tensor("ar_out", [AR_ROWS, D], DT, **ar_kw).ap()

    # cf row map
    CORR0, BIAS0, MM0, TMC0 = 0, 64, 65, 65 + NV * B

    def mmb(out, lhsT, rhs, start, stop, perf_mode=None):
        nc.tensor.matmul(out, lhsT, rhs, start=start, stop=stop,
                         perf_mode=perf_mode)

    with tile.TileContext(nc) as tc:
      with tc.tile_pool(name="const", bufs=1) as cp:
        kern_sb = cp.tile([128, 2 * 128], F8, tag="kern")
        nc.sync.dma_start(
            kern_sb[:].rearrange("j (g i) -> j g i", i=128),
            xk_d[T // 2:T // 2 + 256, 0:128].rearrange(
                "(g j) i -> j g i", g=2))
        ident = cp.tile([128, 128], DT, tag="ident")
        make_identity(nc, ident[:])
        ident8 = cp.tile([128, 128], F8, tag="ident8")
        nc.scalar.copy(ident8[:], ident[:])
        ones_raw = cp.tile([128, 1], DT, tag="ones_raw")
        nc.vector.memset(ones_raw[:], 1.0)
        ones = cp.tile([128, 1], DT, tag="ones")
        nc.scalar.copy(ones[:].bitcast(F32R), ones_raw[:])
        tmc = cp.tile([128, n_aff], DT, tag="tmc")
        nc.sync.dma_start(tmc[:], cf_d[TMC0:TMC0 + 128, 0:n_aff])
        corr_sb = cp.tile([64, D], DT, tag="corr")

        def kslice(v):
            return kern_sb[:, v * 128:(v + 1) * 128]

        with tc.tile_pool(name="fps", bufs=1, space="PSUM") as fps:
            nc.sync.dma_start(corr_sb[:], cf_d[CORR0:CORR0 + 64, :])

            # ---------- fused main loop over t-block pairs ----------
            f1_ps = fps.tile([128, D], DT, tag="f1")   # x | s1*n1
            f2_ps = fps.tile([128, D], DT, tag="f2")   # dist | zt + s2*n2
            with (
                tc.tile_pool(name="pc_sb", bufs=1) as pc,
                tc.tile_pool(name="pc_ps", bufs=1, space="PSUM") as pcps,
            ):
                for tp in range(TP):
                    rows = slice(tp * 128, (tp + 1) * 128)
                    # nf: [128, (i, w=x|s1n1|s2n2, b, c)] fp8, one DMA
                    nf = pc.tile([128, 6 * R], F8, tag="nf", bufs=2)
                    nc.sync.dma_start(nf[:], xk_d[rows, :])
                    # pz2: [128, (i, v=dist|comb, b, c)] fp8
                    pz2 = pc.tile([128, 4 * R], F8, tag="pz", bufs=2)
                    for i in range(2):
                        tb = 2 * tp + i
                        a = tb_aff.index(tb) if tb in tb_aff else None
                        xsl = nf[:, i * 3 * R:i * 3 * R + R]
                        n2sl = nf[:, i * 3 * R + 2 * R:i * 3 * R + 3 * R]
                        cdst = pz2[:, (2 * i + 1) * R:(2 * i + 2) * R]
                        for rc in range(RC):
                            sl = slice(rc * RW, (rc + 1) * RW)
                            # combined view: circ(x) (+ s2*n2 in psum on the
                            # unmasked fast path), then -> fp8
                            zc = pcps.tile([128, RW], DT, tag="zc", bufs=2)
                            if a is None:
                                mmb(zc[:], kslice(1), xsl[:, sl], True, False)
                                mmb(zc[:], ident8[:], n2sl[:, sl],
                                    False, True)
                                nc.vector.tensor_scalar_mul(
                                    cdst[:, sl], zc[:], 1.0)
                            else:
                                mmb(zc[:], kslice(1), xsl[:, sl], True, True)
                                ztv = pc.tile([128, RW], BF, tag="ztv",
                                              bufs=2)
                                nc.vector.tensor_scalar_mul(
                                    ztv[:], zc[:], tmc[:, a:a + 1])
                                n2b = pc.tile([128, RW], BF, tag="n2b",
                                              bufs=2)
                                nc.scalar.copy(n2b[:], n2sl[:, sl])
                                nc.vector.tensor_add(ztv[:], ztv[:], n2b[:])
                                nc.scalar.copy(cdst[:, sl], ztv[:])
                            # distorted view -> fp8 (i, v=0)
                            zd = pcps.tile([128, RW], DT, tag="zd", bufs=2)
                            mmb(zd[:], kslice(0), xsl[:, sl], True, True)
                            dst = pz2[:, 2 * i * R + rc * RW:
                                      2 * i * R + (rc + 1) * RW]
                            if a is not None:
                                nc.vector.tensor_scalar_mul(
                                    dst, zd[:], tmc[:, a:a + 1])
                            else:
                                nc.scalar.copy(dst, zd[:])
                    # feature matmuls: DoubleRow over the (i) pair
                    for g in range(G):
                        wb = pc.tile([128, GC * 2 * D], F8, tag="w", bufs=3)
                        nc.sync.dma_start(
                            wb[:],
                            w_d[rows, g * GC * 2 * D:(g + 1) * GC * 2 * D])
                        for c8 in range(GC):
                            cl = g * GC + c8
                            st = tp == 0 and cl == 0
                            sp = tp == TP - 1 and cl == CH - 1
                            # rhs: [128, (i, d)] for channel cl
                            wsl = wb[:].rearrange(
                                "p (cc i d) -> p cc i d",
                                cc=GC, i=2)[:, c8]
                            lhs1 = nf[:].rearrange(
                                "p (i w b c) -> p i w b c",
                                i=2, w=3, c=CH)[:, :, 0:2, :, cl]
                            lhs2 = pz2[:].rearrange(
                                "p (i v b c) -> p i v b c",
                                i=2, v=2, c=CH)[:, :, :, :, cl]
                            mmb(f1_ps[:], lhs1, wsl, st, sp, perf_mode=DR)
                            mmb(f2_ps[:], lhs2, wsl, st, sp, perf_mode=DR)

            # ---------- all-reduce partial features ----------
            with tc.tile_pool(name="pd_sb", bufs=1) as pd:
                fsb = [pd.tile([128, D], DT, tag=f"fsb{i}", name=f"fsb{i}")
                       for i in range(2)]
                nc.scalar.copy(fsb[0][:], f1_ps[:])
                nc.scalar.copy(fsb[1][:], f2_ps[:])
                # ar rows: 0:64 x, 64:128 s1n1, 128:192 dist, 192:256 comb
                nc.gpsimd.dma_start(ar_in[0:128], fsb[0][:])
                nc.gpsimd.dma_start(ar_in[128:256], fsb[1][:])
                if use_collective:
                    nc.gpsimd.collective_compute(
                        "AllReduce", mybir.AluOpType.add,
                        replica_groups=[list(range(n_cores))],
                        ins=[ar_in], outs=[ar_out])
                else:
                    nc.gpsimd.dma_start(ar_out, ar_in)

        # ---------- loss tail (identical on every core) ----------
        with (
            tc.tile_pool(name="pe_sb", bufs=1) as pe,
            tc.tile_pool(name="pe_ps", bufs=1, space="PSUM") as peps,
        ):
            bb = pe.tile([128, D], DT, tag="bb")
            nc.gpsimd.dma_start(
                out=bb[:], in_=cf_d[BIAS0:BIAS0 + 1, :].to_broadcast((128, D)))
            ag = [pe.tile([64, D], DT, tag=f"ag{v}", name=f"ag{v}")
                  for v in range(4)]
            for v in range(4):
                nc.sync.dma_start(ag[v][:], ar_out[v * 64:(v + 1) * 64])
            # features (dict order): x, compressed, distorted, noisy, combined
            fv = [pe.tile([64, D], DT, tag=f"fv{v}", name=f"fv{v}")
                  for v in range(NV)]
            nc.vector.tensor_add(fv[0][:], ag[0][:], bb[0:64, :])      # x
            nc.vector.tensor_scalar_mul(fv[1][:], ag[0][:], 0.5)       # comp
            nc.vector.tensor_add(fv[1][:], fv[1][:], corr_sb[:])
            nc.vector.tensor_add(fv[1][:], fv[1][:], bb[0:64, :])
            nc.vector.tensor_add(fv[2][:], ag[2][:], bb[0:64, :])      # dist
            nc.vector.tensor_add(fv[3][:], ag[0][:], ag[1][:])         # noisy
            nc.vector.tensor_add(fv[3][:], fv[3][:], bb[0:64, :])
            nc.vector.tensor_add(fv[4][:], ag[3][:], bb[0:64, :])      # comb

            # consistency: sum over v of ||f0 - fv||^2
            cacc = pe.tile([64, 4], DT, tag="cacc")
            for v in range(1, NV):
                dd = pe.tile([64, D], DT, tag="dd", bufs=2)
                nc.vector.tensor_sub(dd[:], fv[v][:], fv[0][:])
                dsq = pe.tile([64, D], DT, tag="dsq", bufs=2)
                nc.scalar.activation(dsq[:], dd[:], AF.Square,
                                     accum_out=cacc[:, v - 1:v])
            cps = peps.tile([1, 4], DT, tag="smallps")
            nc.tensor.matmul(cps[:], ones[0:64, :], cacc[:],
                             start=True, stop=True)
            csb = pe.tile([1, 4], DT, tag="csb")
            nc.scalar.copy(csb[:], cps[:])
            cons = pe.tile([1, 1], DT, tag="cons")
            nc.vector.tensor_reduce(cons[:], csb[:], mybir.AxisListType.X,
                                    mybir.AluOpType.add)

            # normalize rows
            for v in range(NV):
                nrm = pe.tile([64, 1], DT, tag="nrm", bufs=2)
                scr = pe.tile([64, D], DT, tag="scr", bufs=2)
                nc.scalar.activation(scr[:], fv[v][:], AF.Square,
                                     accum_out=nrm[:])
                nc.scalar.sqrt(nrm[:], nrm[:])
                rnr = pe.tile([64, 1], DT, tag="rnr", bufs=2)
                nc.vector.reciprocal(rnr[:], nrm[:])
                nc.vector.tensor_scalar_mul(fv[v][:], fv[v][:], rnr[:])

            # fnT [d-part, 320]
            fnT = [pe.tile([128, NV * B], DT, tag=f"fnT{dc}", name=f"fnT{dc}")
                   for dc in range(4)]
            for v in range(NV):
                for dc in range(4):
                    tp_ = peps.tile([128, 64], DT, tag="ttr", bufs=2)
                    nc.tensor.transpose(
                        tp_[:], fv[v][:, dc * 128:(dc + 1) * 128],
                        ident[0:64, 0:64])
                    nc.scalar.copy(fnT[dc][:, v * 64:(v + 1) * 64], tp_[:])

            # sim rows, logsumexp (no max shift: |sim/T|<=10), masked sums
            mrow = [0, 128, 256]
            mlen = [128, 128, 64]
            parts = []
            for rk in range(3):
                n_r = mlen[rk]
                sps = peps.tile([n_r, NV * B], DT, tag="sps", bufs=2)
                for dc in range(4):
                    lhs = fnT[dc][:, mrow[rk]:mrow[rk] + n_r]
                    nc.tensor.matmul(sps[:], lhs, fnT[dc][:],
                                     start=dc == 0, stop=dc == 3)
                sim = pe.tile([n_r, NV * B], DT, tag=f"sim{rk}")
                nc.scalar.copy(sim[:], sps[:])
                esc = pe.tile([n_r, NV * B], DT, tag="esc", bufs=2)
                sume = pe.tile([n_r, 1], DT, tag="sume", bufs=2)
                nc.scalar.activation(esc[:], sim[:], AF.Exp,
                                     scale=10.0, accum_out=sume[:])
                lse = pe.tile([n_r, 1], DT, tag="lse", bufs=2)
                nc.scalar.activation(lse[:], sume[:], AF.Ln)
                # masked raw sum
                mmt = pe.tile([n_r, NV * B], DT, tag="mmt", bufs=2)
                nc.sync.dma_start(
                    mmt[:], cf_d[MM0 + mrow[rk]:MM0 + mrow[rk] + n_r, 0:320])
                nc.vector.tensor_mul(mmt[:], mmt[:], sim[:])
                mr = pe.tile([n_r, 1], DT, tag="mr", bufs=2)
                nc.vector.tensor_reduce(mr[:], mmt[:], mybir.AxisListType.X,
                                        mybir.AluOpType.add)
                nc.vector.tensor_scalar_mul(mr[:], mr[:], 10.0)
                cntt = pe.tile([n_r, 1], DT, tag="cntt", bufs=2)
                nc.sync.dma_start(
                    cntt[:],
                    cf_d[MM0 + mrow[rk]:MM0 + mrow[rk] + n_r, 320:321])
                nc.vector.tensor_mul(cntt[:], cntt[:], lse[:])
                nc.vector.tensor_sub(mr[:], mr[:], cntt[:])
                parts.append(mr)
            stk = pe.tile([128, 3], DT, tag="stk")
            nc.vector.memset(stk[:], 0.0)
            nc.scalar.copy(stk[:, 0:1], parts[0][:])
            nc.scalar.copy(stk[:, 1:2], parts[1][:])
            nc.scalar.copy(stk[0:64, 2:3], parts[2][:])
            mps = peps.tile([1, 3], DT, tag="smallps")
            nc.tensor.matmul(mps[:], ones[:], stk[:], start=True, stop=True)
            msb = pe.tile([1, 3], DT, tag="msb")
            nc.scalar.copy(msb[:], mps[:])
            msum = pe.tile([1, 1], DT, tag="msum")
            nc.vector.tensor_reduce(msum[:], msb[:], mybir.AxisListType.X,
                                    mybir.AluOpType.add)

            # total = cons/(4*B*D) - 0.5 * msum / (2*NV*B - 2)
            nc.scalar.mul(cons[:], cons[:], 1.0 / (4 * B * D))
            nc.scalar.mul(msum[:], msum[:], -0.5 / float(2 * NV * B - 2))
            tot = pe.tile([1, 1], DT, tag="tot")
            nc.vector.tensor_add(tot[:], cons[:], msum[:])
            nc.sync.dma_start(out_d, tot[:])

    nc.compile()
    return nc


def _get_nc(tb_aff, n_cores, use_collective):
    key = (tuple(tb_aff), n_cores, use_collective)
    if key not in _NC_CACHE:
        _NC_CACHE[key] = _build_nc(list(tb_aff), n_cores, use_collective)
    return _NC_CACHE[key]


def make_in_maps(x, W, b, noise1, noise2, freq_start, time_start,
                 n_cores=None):
    import ml_dtypes
    F8 = ml_dtypes.float8_e4m3
    if n_cores is None:
        n_cores = N_CORES
    CH = C // n_cores
    R = B * CH
    kern, tb_aff, tcols, maskmat, cnt, tmask, msym_half = _host_consts(
        int(freq_start), int(time_start))
    n_aff = len(tb_aff)
    x = np.asarray(x, dtype=np.float32)
    W = np.asarray(W, dtype=np.float32)
    b = np.asarray(b, dtype=np.float32)
    xf8 = x.astype(F8)

    # host-side noise scales: s1 from x, s2 from the exact combined view;
    # noise shipped pre-scaled (f(x + s*n) needs s*n only)
    z = np.fft.irfft(np.fft.rfft(x, axis=-1) * msym_half, axis=-1)
    zt = z * tmask
    s1_all = (NOISE_STD * x.std(-1, ddof=1)).astype(np.float32)  # [B, C]
    s2_all = (NOISE_STD * zt.std(-1, ddof=1)).astype(np.float32)
    n1f8 = (np.asarray(noise1, dtype=np.float32)
            * s1_all[..., None]).astype(F8)
    n2f8 = (np.asarray(noise2, dtype=np.float32)
            * s2_all[..., None]).astype(F8)

    # exact compressed-view correction, summed over all channels (fp32)
    Wr = W.reshape(C, T, D)
    WeT = Wr[:, 0::2, :].sum(axis=1) * (1.0 / T)   # [C, D]
    WoT = Wr[:, 1::2, :].sum(axis=1) * (1.0 / T)
    s_e = x[:, :, 0::2].sum(-1)                    # [B, C]
    s_o = x[:, :, 1::2].sum(-1)
    corr_tot = (s_o @ WeT + s_e @ WoT).astype(np.float32)       # [B, D]

    # packed f32 consts: corr | bias | maskmat+cnt | tmaskc
    cf = np.zeros((CF_ROWS, 512), np.float32)
    cf[0:64] = corr_tot
    cf[64, :] = b
    cf[65:65 + NV * B, 0:NV * B] = maskmat
    cf[65:65 + NV * B, NV * B:NV * B + 1] = cnt
    cf[65 + NV * B:65 + NV * B + 128, 0:n_aff] = tcols

    Wf8 = W.astype(F8).reshape(C, T // 256, 2, 128, D)
    kern_f8 = kern.astype(F8)
    in_maps = []
    for core in range(n_cores):
        cs = core * CH
        # x|s1n1|s2n2 [T, 3R] -> pair-interleaved [T/2, 6R] + kern rows
        tri = np.concatenate(
            [t[:, cs:cs + CH, :].reshape(R, T).T
             for t in (xf8, n1f8, n2f8)], axis=1)          # [T, 3R]
        tri = tri.reshape(T // 256, 2, 128, 3 * R).transpose(0, 2, 1, 3) \
                 .reshape(T // 2, 6 * R)
        xk = np.zeros((T // 2 + 256, 6 * R), F8)
        xk[:T // 2] = tri
        xk[T // 2:, 0:128] = kern_f8.reshape(256, 128)
        # W [CH, T/256, 2, 128, D] -> [T/2, (c, i, d)]
        Wc = Wf8[cs:cs + CH].transpose(1, 3, 0, 2, 4) \
                            .reshape(T // 2, CH * 2 * D)
        in_maps.append({
            "xn12k": np.ascontiguousarray(xk),
            "Ws": np.ascontiguousarray(Wc),
            "cf": cf,
        })
    return in_maps, tb_aff


def kernel(x, W, b, noise1, noise2, freq_start, time_start):
    from concourse.bass_utils import run_bass_kernel_spmd
    in_maps, tb_aff = make_in_maps(x, W, b, noise1, noise2,
                                   freq_start, time_start)
    nc = _get_nc(tb_aff, N_CORES, N_CORES > 1)
    res = run_bass_kernel_spmd(nc, in_maps, core_ids=list(range(N_CORES)))
    return np.float32(res.results[0]["out_loss"].reshape(())[()])
